# revision 5
# baseline (speedup 1.0000x reference)
"""Trainium2 Bass kernel for nn_EvolvingLocalConvBlock.

Sharding: 8 cores = 4 samples x 2 sequence halves (1024 tokens each).
Cross-core cumsum carries are exchanged via three pairwise AllReduces
(even core sends masked half-totals; odd core consumes them).

On-chip layout is feature-major (FM): SBUF tiles are (128 features, T tokens).
Linears run on PE in fp32r; L-cumsums run on DVE tensor_tensor_scan along the
free dim; transcendentals on ACT batched by table set (gelu -> ln/exp -> trig
-> gelu).  Cross-phase tensors (phi, mem1 scans, LN pieces, ...) are staged in
DRAM to fit SBUF; SBUF tags are shared between tensors with disjoint
lifetimes.
"""
import sys
sys.path.insert(0, '/opt/trn_rl_repo')

import math
import numpy as np

import concourse.bass as bass
import concourse.mybir as mybir
from concourse.tile import TileContext

B, L, D, P, V, K = 4, 2048, 512, 128, 8, 4
N_CORES = 8
NT = L // 2
NCH = NT // 128
ND = D // 128
NBLK = 2
TB = NT // NBLK

f32 = mybir.dt.float32
f32r = mybir.dt.float32r
A = mybir.ActivationFunctionType
Alu = mybir.AluOpType

TWO_PI = 2.0 * math.pi
HALF_PI = math.pi / 2.0

# ---- bias_pack column map ----
BC = {}
_ncols = 0
def _bc(name, n):
    global _ncols
    BC[name] = _ncols
    _ncols += n
for _n, _k in [("tw_b", ND), ("pi0_b", ND), ("pi2_b", ND), ("m1v_b", ND),
               ("mag_b", ND), ("qo_b", ND), ("cp_b", ND), ("m1o_b", ND),
               ("ke_b", 1), ("ve_b", 1), ("sg_b", 1), ("sk0_b", ND),
               ("sk2_b", 1), ("kvo_b", ND), ("o1_b", 8), ("negw", 8),
               ("lc_b", ND), ("cg_b", ND),
               ("lc_w", ND * K), ("cg_w", ND * K),
               ("halfpi", 1), ("eps_mag", 1), ("c_mag", 1), ("eps_ln", 1)]:
    _bc(_n, _k)
NBIAS = _ncols


def fixup_excess_waits(nc, max_waits=1, max_updates=1):
    """This walrus accepts at most one sync wait/update per instruction;
    hoist extras onto adjacent same-engine NoOps."""
    for f in nc.m.functions:
        for bb in f.blocks:
            new = []
            changed = False
            for ins in bb.instructions:
                si = getattr(ins, 'sync_info', None)
                if si is None:
                    new.append(ins)
                    continue
                w = list(si.on_wait) if si.on_wait else []
                if len(w) > max_waits:
                    excess, keep = w[:-max_waits], w[-max_waits:]
                    for i in range(0, len(excess), max_waits):
                        nop = mybir.InstNoOp(name=f"{ins.name}-hw{i}",
                                             engine=ins.engine, ins=[], outs=[])
                        nop.sync_info = mybir.SyncInfo(
                            on_wait=excess[i:i + max_waits], on_update=[])
                        new.append(nop)
                    si.on_wait = keep
                    changed = True
                new.append(ins)
                u = list(si.on_update) if si.on_update else []
                if len(u) > max_updates:
                    excess_u, keep_u = u[max_updates:], u[:max_updates]
                    for i in range(0, len(excess_u), max_updates):
                        nop = mybir.InstNoOp(name=f"{ins.name}-hu{i}",
                                             engine=ins.engine, ins=[], outs=[])
                        nop.sync_info = mybir.SyncInfo(
                            on_wait=[], on_update=excess_u[i:i + max_updates])
                        new.append(nop)
                    si.on_update = keep_u
                    changed = True
            if changed:
                bb.instructions = new


def build_nc(debug=()):
    import concourse.tile_utils as tile_utils
    tile_utils.max_sbuf_usage = 204 * 1024

    nc = bass.Bass()
    dp = nc.declare_dram_parameter

    x_ext = dp("x_ext", [NT + 3, D], f32, isOutput=False)
    y_out = dp("y", [NT, D], f32, isOutput=True)

    wts = {}
    for name, shape in [
        ("wT_tw", [D, D]), ("wT_pi0", [D, D]), ("wT_pi2", [D, D]),
        ("wT_m1v", [D, D]), ("wT_mag", [D, D]), ("wT_qo", [D, D]),
        ("wT_cp", [D, D]), ("wT_m1o", [D, D]),
        ("kepack", [128, ND * 128]), ("vepack", [128, ND * V]),
        ("sgpack", [128, ND]), ("wT_sk0", [2 * D, D]),
        ("sk2pack", [128, ND * 128]), ("wT_kvo", [V, D]),
        ("wT_o1", [5 * D, 2 * D]), ("wT_o2", [2 * D, D]),
        ("negWsum", [1, 2 * D]), ("o2b_row", [1, D]),
        ("ones_col", [128, 1]), ("ones_row1", [1, 128]),
        ("eye_r", [128, 128]),
    ]:
        wts[name] = dp(name, shape, f32r, isOutput=False)
    eye_f = dp("eye_f", [128, 128], f32, isOutput=False)
    tril_in = dp("tril", [128, 128], f32, isOutput=False)
    bias_in = dp("bias_pack", [128, NBIAS], f32, isOutput=False)
    recip_in = dp("recip_pos", [128, NT], f32, isOutput=False)
    smask_in = dp("send_mask", [128, 1], f32, isOutput=False)
    umask_in = dp("use_mask", [128, 1], f32, isOutput=False)

    dbg_shapes = {}
    RG = [[0, 1], [2, 3], [4, 5], [6, 7]]

    with TileContext(nc) as tc:
        # LIFO pool stacks.  left: con, wleft.  right: big -> p9.
        con = tc.alloc_tile_pool(name="con", bufs=1, side="left")
        wleft = tc.alloc_tile_pool(name="wleft", bufs=6, side="left")
        big = tc.alloc_tile_pool(name="big", bufs=1)
        pb = tc.alloc_tile_pool(name="pb", bufs=4, space="PSUM")
        psm = tc.alloc_tile_pool(name="psm", bufs=2, space="PSUM")
        dram = tc.alloc_tile_pool(name="dram", bufs=1, space="DRAM")

        dbg_bufs = {}
        def dbg(name, ap, part):
            """Dump (rows, NT) AP into 128-row slot `part` of a debug out."""
            if name not in debug:
                return
            r = ap.shape[0]
            if name not in dbg_bufs:
                dbg_bufs[name] = dp("dbg_" + name, [ND * 128, NT], f32,
                                    isOutput=True)
                dbg_shapes[name] = True
            t = dbg_bufs[name]
            if ap.dtype == f32r:
                ap = ap.bitcast(f32)
            nc.sync.dma_start(out=t[128 * part:128 * part + r, :], in_=ap)

        # ---------------- constants ----------------
        bias = con.tile([128, NBIAS], f32, tag="bias")
        nc.sync.dma_start(out=bias[:], in_=bias_in[:])
        def bc(name, i=0, rows=128):
            return bias[0:rows, BC[name] + i:BC[name] + i + 1]
        eyef = con.tile([128, 128], f32, tag="eyef")
        nc.sync.dma_start(out=eyef[:], in_=eye_f[:])
        eyer = con.tile([128, 128], f32r, tag="eyer")
        nc.sync.dma_start(out=eyer[:], in_=wts["eye_r"][:])
        trilm = con.tile([128, 128], f32, tag="tril")
        nc.sync.dma_start(out=trilm[:], in_=tril_in[:])
        smask = con.tile([128, 1], f32, tag="smask")
        nc.sync.dma_start(out=smask[:], in_=smask_in[:])
        umask = con.tile([128, 1], f32, tag="umask")
        nc.sync.dma_start(out=umask[:], in_=umask_in[:])
        onesr = con.tile([128, 1], f32r, tag="onesr")
        nc.sync.dma_start(out=onesr[:], in_=wts["ones_col"][:])
        ones_r1 = con.tile([1, 128], f32r, tag="onesr1")
        nc.sync.dma_start(out=ones_r1[:], in_=wts["ones_row1"][:])
        zeros = con.tile([128, NT], f32, tag="zeros")
        nc.vector.memset(zeros[:], 0.0)

        # ---------------- DRAM staging ----------------
        def dstage(tag, dt=f32):
            return dram.tile([ND * 128, NT], dt, tag=tag, name=tag)
        d_Som = dstage("dSom")
        d_phi = dstage("dphi")
        d_phiq = dstage("dphiq")
        d_Sc = dstage("dSc")
        d_Ss = dstage("dSs")
        d_cosq = dstage("dcosq")
        d_sinq = dstage("dsinq")
        d_posret = dstage("dposret", f32r)
        d_pc = [dstage(f"dpc{i}", f32r) for i in range(5)]  # conv,pos,kv,xc,xs

        # ---------------- P0: x load + transpose to FM ----------------
        x_fm = []
        xl_tm = big.tile([3, D], f32, tag="xtm")
        nc.sync.dma_start(out=xl_tm[:], in_=x_ext[0:3, :])
        for d in range(ND):
            xt = big.tile([128, NT + 3], f32r, tag=f"x{d}")
            ps = psm.tile([128, 3], f32, tag="tr")
            nc.tensor.transpose(ps[:], xl_tm[:, 128 * d:128 * (d + 1)],
                                eyef[0:3, 0:3])
            nc.scalar.copy(xt[:, 0:3], ps[:])
            x_fm.append(xt)
        for j in range(NCH):
            t = big.tile([128, D], f32, tag="xtm")
            nc.sync.dma_start(out=t[:],
                              in_=x_ext[3 + 128 * j:3 + 128 * (j + 1), :])
            for d in range(ND):
                ps2 = psm.tile([128, 128], f32, tag="tr")
                nc.tensor.transpose(ps2[:], t[:, 128 * d:128 * (d + 1)],
                                    eyef[:])
                nc.scalar.copy(x_fm[d][:, 3 + 128 * j:3 + 128 * (j + 1)],
                               ps2[:])
        xin = [xt[:, 3:3 + NT] for xt in x_fm]

        # ---------------- helpers ----------------
        def load_wrows(name, nin, nout, tag="w4", bufs=4):
            rows = []
            for i in range(nin):
                t = wleft.tile([128, nout], f32r, tag=tag, bufs=bufs,
                               name=f"{name}r{i}")
                nc.sync.dma_start(out=t[:],
                                  in_=wts[name][128 * i:128 * (i + 1), :])
                rows.append(t)
            return rows

        def mm_big(wname, rhs_tiles, epilogue, nout=D):
            """epilogue(o, blk, psum (128,TB))"""
            rows = load_wrows(wname, len(rhs_tiles), nout)
            for blk in range(NBLK):
                cs = slice(TB * blk, TB * (blk + 1))
                for o in range(nout // 128):
                    ps = pb.tile([128, TB], f32, tag="lin")
                    for i, r in enumerate(rhs_tiles):
                        nc.tensor.matmul(ps[:],
                                         rows[i][:, 128 * o:128 * (o + 1)],
                                         r[:, cs], start=(i == 0),
                                         stop=(i == len(rhs_tiles) - 1))
                    epilogue(o, blk, ps)

        def mm_packed(wname, rhs_tiles, out_rows, epilogue):
            """packed weight (128, nin*out_rows); epilogue(blk, ps)."""
            nin = len(rhs_tiles)
            wrow = wleft.tile([128, nin * out_rows], f32r, tag="wp1",
                              bufs=2, name=wname)
            nc.sync.dma_start(out=wrow[:], in_=wts[wname][:])
            for blk in range(NBLK):
                cs = slice(TB * blk, TB * (blk + 1))
                ps = pb.tile([out_rows, TB], f32, tag="lin")
                for i in range(nin):
                    nc.tensor.matmul(ps[:],
                                     wrow[:, out_rows * i:out_rows * (i + 1)],
                                     rhs_tiles[i][:, cs],
                                     start=(i == 0), stop=(i == nin - 1))
                epilogue(blk, ps)

        def scan_full(dst_ap, src_ap, rows=128):
            nc.vector.tensor_tensor_scan(dst_ap, zeros[0:rows, 0:NT], src_ap,
                                         0.0, Alu.add, Alu.add)

        def exchange(n, fill):
            pk = big.tile([128, n], f32, tag="pk")
            nc.vector.memset(pk[:], 0.0)
            fill(pk)
            cin = dram.tile([128, n], f32, tag=f"ci{n}")
            cout = dram.tile([128, n], f32, tag=f"co{n}")
            nc.sync.dma_start(out=cin[:], in_=pk[:])
            nc.gpsimd.collective_compute(
                "AllReduce", Alu.add, replica_groups=RG,
                ins=[cin.opt()], outs=[cout.opt()])
            rcv = big.tile([128, n], f32, tag=f"rc{n}")
            nc.sync.dma_start(out=rcv[:], in_=cout[:])
            rcvu = big.tile([128, n], f32, tag=f"ru{n}")
            nc.vector.tensor_scalar(rcvu[:], rcv[:], umask[:, 0:1], None,
                                    Alu.mult)
            return rcvu

        # ======== P2a: conv branch (DVE MACs, tanh, cp linear) ========
        convg = []
        for d in range(ND):
            co = big.tile([128, NT], f32, tag="convco")
            nc.vector.tensor_scalar(co[:], x_fm[d][:, 0:NT],
                                    bc("lc_w", 4 * d + 0), bc("lc_b", d),
                                    Alu.mult, Alu.add)
            for k in range(1, K):
                nc.vector.scalar_tensor_tensor(
                    co[:], x_fm[d][:, k:k + NT], bc("lc_w", 4 * d + k), co[:],
                    Alu.mult, Alu.add)
            cg = big.tile([128, NT], f32, tag="convcg")
            nc.vector.tensor_scalar(cg[:], x_fm[d][:, 0:NT],
                                    bc("cg_w", 4 * d + 0), bc("cg_b", d),
                                    Alu.mult, Alu.add)
            for k in range(1, K):
                nc.vector.scalar_tensor_tensor(
                    cg[:], x_fm[d][:, k:k + NT], bc("cg_w", 4 * d + k), cg[:],
                    Alu.mult, Alu.add)
            nc.scalar.activation(cg[:], cg[:], A.Tanh, scale=0.5)
            nc.vector.tensor_scalar(cg[:], cg[:], 0.5, 0.5, Alu.mult, Alu.add)
            gt = big.tile([128, NT], f32r, tag=f"tA{d}")   # shared tag A
            nc.vector.tensor_tensor(gt[:], cg[:], co[:], Alu.mult)
            convg.append(gt)
            dbg("convg", gt[:], d)

        def ep_store(dtensor, bname):
            def ep(o, blk, ps):
                t = big.tile([128, TB], f32r, tag="xtm")
                nc.scalar.activation(t[:], ps[:], A.Identity, bias=bc(bname, o))
                nc.sync.dma_start(
                    out=dtensor[128 * o:128 * (o + 1), TB * blk:TB * (blk + 1)],
                    in_=t[:])
            return ep
        mm_big("wT_cp", [t[:] for t in convg], ep_store(d_pc[0], "cp_b"))

        # ======== P2b: first-level linears [gelu set] ========
        g0 = [big.tile([128, NT], f32r, tag=f"tB{o}", name=f"g0{o}") for o in range(ND)]
        def ep_g0(o, blk, ps):
            nc.scalar.activation(g0[o][:, TB * blk:TB * (blk + 1)], ps[:],
                                 A.Gelu, bias=bc("pi0_b", o))
        mm_big("wT_pi0", xin, ep_g0)

        v1 = [big.tile([128, NT], f32, tag=f"v1{o}", name=f"v1{o}") for o in range(ND)]
        def ep_v1(o, blk, ps):
            nc.scalar.activation(v1[o][:, TB * blk:TB * (blk + 1)], ps[:],
                                 A.Identity, bias=bc("m1v_b", o))
        mm_big("wT_m1v", xin, ep_v1)

        sig = [big.tile([128, NT], f32, tag=f"sig{o}", name=f"sig{o}") for o in range(ND)]
        def ep_sig(o, blk, ps):
            ap = sig[o][:, TB * blk:TB * (blk + 1)]
            nc.scalar.activation(ap, ps[:], A.Tanh, bias=bc("mag_b", o),
                                 scale=0.5)
            nc.vector.tensor_scalar(ap, ap, 0.5, 0.5, Alu.mult, Alu.add)
        mm_big("wT_mag", xin, ep_sig)
        for d in range(ND):
            dbg("sig", sig[d][:], d)

        # omega -> scan -> spill S_om; collect last cols in `lastc`
        lastc = big.tile([128, 12], f32, tag="lastc")
        omq = [None]
        def ep_om(o, blk, ps):
            if blk == 0:
                omq[0] = big.tile([128, NT], f32, tag="omrot")
            nc.scalar.activation(omq[0][:, TB * blk:TB * (blk + 1)], ps[:],
                                 A.Identity, bias=bc("tw_b", o))
            if blk == NBLK - 1:
                som = big.tile([128, NT], f32, tag="convcg")
                scan_full(som[:], omq[0][:])
                nc.sync.dma_start(out=d_Som[128 * o:128 * (o + 1), :],
                                  in_=som[:])
                nc.vector.tensor_copy(lastc[:, o:o + 1], som[:, NT - 1:NT])
        # NOTE: mm_big iterates blk-outer; ep_om needs o-outer accumulation.
        # Use per-(o,blk) direct emission instead:
        twrows = load_wrows("wT_tw", ND, D)
        for o in range(ND):
            omt = big.tile([128, NT], f32, tag="convco")
            for blk in range(NBLK):
                cs = slice(TB * blk, TB * (blk + 1))
                ps = pb.tile([128, TB], f32, tag="lin")
                for i in range(ND):
                    nc.tensor.matmul(ps[:], twrows[i][:, 128 * o:128 * (o + 1)],
                                     xin[i][:, cs], start=(i == 0),
                                     stop=(i == ND - 1))
                nc.scalar.activation(omt[:, cs], ps[:], A.Identity,
                                     bias=bc("tw_b", o))
            som = big.tile([128, NT], f32, tag="convcg")
            scan_full(som[:], omt[:])
            nc.sync.dma_start(out=d_Som[128 * o:128 * (o + 1), :], in_=som[:])
            nc.vector.tensor_copy(lastc[:, o:o + 1], som[:, NT - 1:NT])

        t_ke = big.tile([128, NT], f32, tag="tke")
        def ep_ke(blk, ps):
            nc.scalar.activation(t_ke[:, TB * blk:TB * (blk + 1)], ps[:],
                                 A.Tanh, bias=bc("ke_b"))
        mm_packed("kepack", xin, 128, ep_ke)

        vals = big.tile([V, NT], f32, tag="vals")
        def ep_ve(blk, ps):
            nc.scalar.activation(vals[:, TB * blk:TB * (blk + 1)], ps[:],
                                 A.Identity, bias=bc("ve_b", rows=V))
        mm_packed("vepack", xin, V, ep_ve)

        gate = big.tile([1, NT], f32, tag="gate")
        def ep_sg(blk, ps):
            ap = gate[:, TB * blk:TB * (blk + 1)]
            nc.scalar.activation(ap, ps[:], A.Tanh, bias=bc("sg_b", rows=1),
                                 scale=0.5)
            nc.vector.tensor_scalar(ap, ap, 0.5, 0.5, Alu.mult, Alu.add)
        mm_packed("sgpack", xin, 1, ep_sg)

        # scans of sigma, x, gate
        S_sig = []
        for d in range(ND):
            t = big.tile([128, NT], f32, tag=f"tA{d}")   # reuse conv tag
            scan_full(t[:], sig[d][:])
            nc.vector.tensor_copy(lastc[:, 4 + d:5 + d], t[:, NT - 1:NT])
            S_sig.append(t)
        S_x = []
        for d in range(ND):
            t = big.tile([128, NT], f32, tag=f"sx{d}")
            scan_full(t[:], xin[d])
            nc.vector.tensor_copy(lastc[:, 8 + d:9 + d], t[:, NT - 1:NT])
            S_x.append(t)
        S_gate = big.tile([1, NT], f32, tag="sgate")
        scan_full(S_gate[:], gate[:], rows=1)

        def fill1(pk):
            for c in range(12):
                nc.vector.tensor_scalar(pk[:, c:c + 1], lastc[:, c:c + 1],
                                        smask[:, 0:1], None, Alu.mult)
            nc.vector.tensor_scalar(pk[0:1, 12:13], S_gate[:, NT - 1:NT],
                                    smask[0:1, 0:1], None, Alu.mult)
        rcv1 = exchange(13, fill1)

        # ======== P3: phi & phi_q; ctx -> sk0 -> sk2 [gelu set] ========
        romb = big.tile([128, ND], f32, tag="romb")
        for d in range(ND):
            nc.vector.tensor_tensor(romb[:, d:d + 1], rcv1[:, d:d + 1],
                                    bc("pi2_b", d), Alu.add)
        pi2rows = load_wrows("wT_pi2", ND, D)
        qorows = load_wrows("wT_qo", ND, D, tag="w8", bufs=8)
        for o in range(ND):
            for blk in range(NBLK):
                cs = slice(TB * blk, TB * (blk + 1))
                psA = pb.tile([128, TB], f32, tag="lin")
                for i in range(ND):
                    nc.tensor.matmul(psA[:],
                                     pi2rows[i][:, 128 * o:128 * (o + 1)],
                                     g0[i][:, cs], start=(i == 0),
                                     stop=(i == ND - 1))
                psB = pb.tile([128, TB], f32, tag="lin")
                for i in range(ND):
                    nc.tensor.matmul(psB[:],
                                     qorows[i][:, 128 * o:128 * (o + 1)],
                                     xin[i][:, cs], start=(i == 0),
                                     stop=(i == ND - 1))
                somc = big.tile([128, TB], f32, tag="xtm")
                nc.sync.dma_start(out=somc[:],
                                  in_=d_Som[128 * o:128 * (o + 1), cs])
                phic = big.tile([128, TB], f32, tag="phic")
                nc.vector.scalar_tensor_tensor(phic[:], somc[:],
                                               romb[:, o:o + 1], psA[:],
                                               Alu.add, Alu.add)
                nc.sync.dma_start(out=d_phi[128 * o:128 * (o + 1), cs],
                                  in_=phic[:])
                phiqc = big.tile([128, TB], f32, tag="phiqc")
                nc.vector.tensor_tensor(phiqc[:], phic[:], psB[:], Alu.add)
                nc.vector.tensor_scalar(phiqc[:], phiqc[:], bc("qo_b", o),
                                        None, Alu.add)
                nc.sync.dma_start(out=d_phiq[128 * o:128 * (o + 1), cs],
                                  in_=phiqc[:])

        # ctx chunks feed sk0 directly
        sk0rows = load_wrows("wT_sk0", 2 * ND, D, tag="w8", bufs=8)
        gsk = [big.tile([128, NT], f32r, tag=f"gsk{o}", name=f"gsk{o}") for o in range(ND)]
        for blk in range(NBLK):
            cs = slice(TB * blk, TB * (blk + 1))
            ctxc = []
            for d in range(ND):
                rpc = big.tile([128, TB], f32, tag="phic2")
                nc.sync.dma_start(out=rpc[:], in_=recip_in[:, cs])
                t = big.tile([128, TB], f32r, tag=f"ctxc{d}")
                nc.vector.tensor_scalar(t[:], S_x[d][:, cs],
                                        rcv1[:, 8 + d:9 + d], None, Alu.add)
                nc.vector.tensor_tensor(t[:], t[:], rpc[:], Alu.mult)
                ctxc.append(t)
            for o in range(ND):
                ps = pb.tile([128, TB], f32, tag="lin")
                for i in range(ND):
                    nc.tensor.matmul(ps[:],
                                     sk0rows[i][:, 128 * o:128 * (o + 1)],
                                     xin[i][:, cs], start=(i == 0), stop=False)
                for i in range(ND):
                    nc.tensor.matmul(
                        ps[:], sk0rows[ND + i][:, 128 * o:128 * (o + 1)],
                        ctxc[i][:], start=False, stop=(i == ND - 1))
                nc.scalar.activation(gsk[o][:, cs], ps[:], A.Gelu,
                                     bias=bc("sk0_b", o))
        t_sk = big.tile([128, NT], f32, tag="tsk")
        def ep_sk2(blk, ps):
            nc.scalar.activation(t_sk[:, TB * blk:TB * (blk + 1)], ps[:],
                                 A.Tanh, bias=bc("sk2_b"))
        mm_packed("sk2pack", [t[:] for t in gsk], 128, ep_sk2)

        # ======== P3.5 [ln/exp set]: rstd_mag, gate norm ========
        rstd_mag = []
        for d in range(ND):
            t = big.tile([128, NT], f32, tag=f"tB{d}")   # reuse g0 tag
            nc.vector.tensor_scalar(t[:], S_sig[d][:], rcv1[:, 4 + d:5 + d],
                                    None, Alu.add)
            nc.scalar.activation(t[:], t[:], A.Ln, bias=bc("eps_mag"),
                                 scale=bc("c_mag"))
            nc.scalar.activation(t[:], t[:], A.Exp, scale=-0.5)
            rstd_mag.append(t)
            dbg("rstdm", t[:], d)
        gn_row = S_gate
        nc.vector.tensor_scalar(gn_row[:], S_gate[:], rcv1[0:1, 12:13],
                                None, Alu.add)
        nc.vector.tensor_scalar(gn_row[:], gn_row[:], 1.0, None, Alu.max)
        nc.scalar.activation(gn_row[:], gn_row[:], A.Ln)
        nc.scalar.activation(gn_row[:], gn_row[:], A.Exp, scale=-0.5)
        dbg("gnr", gn_row[:], 0)
        rstd_g_tm = big.tile([128, NCH], f32, tag="rgtm")
        for j in range(NCH):
            ps = psm.tile([128, 1], f32, tag="tr")
            nc.tensor.transpose(ps[:], gn_row[0:1, 128 * j:128 * (j + 1)],
                                eyef[0:1, 0:1])
            nc.scalar.copy(rstd_g_tm[:, j:j + 1], ps[:])

        # ======== P4 [trig set] ========
        def phase_cs(tin, ctag, stag):
            s_t = big.tile([128, NT], f32r, tag=stag)
            nc.scalar.activation(s_t[:], tin[:], A.Sin, scale=math.pi)
            m = big.tile([128, NT], f32, tag="phic")
            nc.vector.tensor_scalar(m[:], tin[:], 0.5, None, Alu.is_gt)
            nc.vector.scalar_tensor_tensor(m[:], m[:], -2.0, tin[:],
                                           Alu.mult, Alu.add)
            c_t = big.tile([128, NT], f32r, tag=ctag)
            nc.scalar.activation(c_t[:], m[:], A.Sin, scale=math.pi,
                                 bias=bc("halfpi"))
            return c_t, s_t
        Qc, Qs = phase_cs(t_ke, "sx0", "sx1")
        Kc, Ks = phase_cs(t_sk, "sx2", "sx3")
        dbg("Qc", Qc[:], 0)
        dbg("Kc", Kc[:], 0)

        for d in range(ND):
            rs = slice(128 * d, 128 * (d + 1))
            phid = big.tile([128, NT], f32, tag="phic2")
            nc.sync.dma_start(out=phid[:], in_=d_phi[rs, :])
            dbg("phi", phid[:], d)
            cphi = big.tile([128, NT], f32, tag="convco")
            nc.scalar.activation(cphi[:], phid[:], A.Sin, bias=bc("halfpi"))
            sphi = big.tile([128, NT], f32, tag="convcg")
            nc.scalar.activation(sphi[:], phid[:], A.Sin)
            xcd = big.tile([128, NT], f32r, tag="tke")
            nc.vector.tensor_tensor(xcd[:], xin[d], cphi[:], Alu.mult)
            nc.sync.dma_start(out=d_pc[3][rs, :], in_=xcd[:])
            xsd = big.tile([128, NT], f32r, tag="tsk")
            nc.vector.tensor_tensor(xsd[:], xin[d], sphi[:], Alu.mult)
            nc.sync.dma_start(out=d_pc[4][rs, :], in_=xsd[:])
            wv = big.tile([128, NT], f32, tag="wv1")
            nc.vector.tensor_tensor(wv[:], sig[d][:], v1[d][:], Alu.mult)
            nc.vector.tensor_tensor(cphi[:], wv[:], cphi[:], Alu.mult)
            tSc = big.tile([128, NT], f32, tag="sgate")
            scan_full(tSc[:], cphi[:])
            nc.sync.dma_start(out=d_Sc[rs, :], in_=tSc[:])
            dbg("Sc", tSc[:], d)
            nc.vector.tensor_copy(lastc[:, 4 + d:5 + d], tSc[:, NT - 1:NT])
            nc.vector.tensor_tensor(sphi[:], wv[:], sphi[:], Alu.mult)
            tSs = big.tile([128, NT], f32, tag="phic")
            scan_full(tSs[:], sphi[:])
            nc.sync.dma_start(out=d_Ss[rs, :], in_=tSs[:])
            nc.vector.tensor_copy(lastc[:, 8 + d:9 + d], tSs[:, NT - 1:NT])
            phiqd = big.tile([128, NT], f32, tag="phiqc")
            nc.sync.dma_start(out=phiqd[:], in_=d_phiq[rs, :])
            sq_t = big.tile([128, NT], f32, tag="tke")
            nc.scalar.activation(sq_t[:], phiqd[:], A.Sin)
            nc.sync.dma_start(out=d_sinq[rs, :], in_=sq_t[:])
            m = big.tile([128, NT], f32, tag="phic")
            nc.vector.tensor_scalar(m[:], phiqd[:], HALF_PI, None, Alu.is_gt)
            nc.vector.scalar_tensor_tensor(phiqd[:], m[:], -TWO_PI, phiqd[:],
                                           Alu.mult, Alu.add)
            cq_t = big.tile([128, NT], f32, tag="tsk")
            nc.scalar.activation(cq_t[:], phiqd[:], A.Sin, bias=bc("halfpi"))
            nc.sync.dma_start(out=d_cosq[rs, :], in_=cq_t[:])
            dbg("cosq", cq_t[:], d)

        def fill2a(pk):
            for c in range(8):
                nc.vector.tensor_scalar(pk[:, c:c + 1], lastc[:, 4 + c:5 + c],
                                        smask[:, 0:1], None, Alu.mult)
        rcv2a = exchange(8, fill2a)

        # ======== combine -> pos_ret -> m1o (streamed) ========
        for d in range(ND):
            rs = slice(128 * d, 128 * (d + 1))
            tSc = big.tile([128, NT], f32, tag="sgate")
            nc.sync.dma_start(out=tSc[:], in_=d_Sc[rs, :])
            cq = big.tile([128, NT], f32, tag="tke")
            nc.sync.dma_start(out=cq[:], in_=d_cosq[rs, :])
            t1 = big.tile([128, NT], f32, tag="wv1")
            nc.vector.scalar_tensor_tensor(t1[:], tSc[:], rcv2a[:, d:d + 1],
                                           cq[:], Alu.add, Alu.mult)
            tSs = big.tile([128, NT], f32, tag="phic")
            nc.sync.dma_start(out=tSs[:], in_=d_Ss[rs, :])
            sq = big.tile([128, NT], f32, tag="tsk")
            nc.sync.dma_start(out=sq[:], in_=d_sinq[rs, :])
            nc.vector.scalar_tensor_tensor(sq[:], tSs[:],
                                           rcv2a[:, 4 + d:5 + d], sq[:],
                                           Alu.add, Alu.mult)
            nc.vector.tensor_tensor(t1[:], t1[:], sq[:], Alu.add)
            pr = big.tile([128, NT], f32r, tag="phic2")
            nc.vector.tensor_tensor(pr[:], t1[:], rstd_mag[d][:], Alu.mult)
            nc.sync.dma_start(out=d_posret[rs, :], in_=pr[:])
            dbg("pos_ret", pr[:], d)

        m1orows = load_wrows("wT_m1o", ND, D)
        for blk in range(NBLK):
            cs = slice(TB * blk, TB * (blk + 1))
            prc = []
            for i in range(ND):
                t = big.tile([128, TB], f32r, tag=f"ctxc{i}")
                nc.sync.dma_start(out=t[:],
                                  in_=d_posret[128 * i:128 * (i + 1), cs])
                prc.append(t)
            for o in range(ND):
                ps = pb.tile([128, TB], f32, tag="lin")
                for i in range(ND):
                    nc.tensor.matmul(ps[:],
                                     m1orows[i][:, 128 * o:128 * (o + 1)],
                                     prc[i][:], start=(i == 0),
                                     stop=(i == ND - 1))
                t = big.tile([128, TB], f32r, tag="xtm")
                nc.scalar.activation(t[:], ps[:], A.Identity,
                                     bias=bc("m1o_b", o))
                nc.sync.dma_start(out=d_pc[1][128 * o:128 * (o + 1), cs],
                                  in_=t[:])

        # ======== P7: kv attention ========
        pkv = tc.alloc_tile_pool(name="pkv", bufs=1, space="PSUM")
        retr_sb = big.tile([128, V * NCH], f32, tag="retr")
        Ccos_sb = big.tile([128, V], f32r, tag="ccos")
        Csin_sb = big.tile([128, V], f32r, tag="csin")
        kvo_w = wleft.tile([V, D], f32r, tag="wk", bufs=1)
        nc.sync.dma_start(out=kvo_w[:], in_=wts["wT_kvo"][:])
        for j in range(NCH):
            ch = slice(128 * j, 128 * (j + 1))
            ps_st = psm.tile([128, 128], f32, tag="tr")
            nc.tensor.matmul(ps_st[:], Kc[:, ch], Qc[:, ch],
                             start=True, stop=False)
            nc.tensor.matmul(ps_st[:], Ks[:, ch], Qs[:, ch],
                             start=False, stop=True)
            st_sb = big.tile([128, 128], f32r, tag="stsb")
            nc.vector.tensor_tensor(st_sb[:], ps_st[:], trilm[:], Alu.mult)
            ps_v = psm.tile([128, V + 1], f32, tag="tr")
            nc.tensor.transpose(ps_v[:, 0:V], vals[:, ch], eyef[0:V, 0:V])
            nc.tensor.transpose(ps_v[:, V:V + 1], gate[0:1, ch],
                                eyef[0:1, 0:1])
            gv = big.tile([128, V], f32r, tag="gv")
            nc.vector.tensor_scalar(gv[:], ps_v[:, 0:V], ps_v[:, V:V + 1],
                                    None, Alu.mult)
            ps_r = pkv.tile([128, V], f32, tag="pr")
            nc.tensor.matmul(ps_r[:], st_sb[:], gv[:], start=True,
                             stop=(j == 0))
            if j > 0:
                nc.tensor.matmul(ps_r[:], Qc[:, ch], Ccos_sb[:],
                                 start=False, stop=False)
                nc.tensor.matmul(ps_r[:], Qs[:, ch], Csin_sb[:],
                                 start=False, stop=True)
            nc.scalar.copy(retr_sb[:, V * j:V * (j + 1)], ps_r[:])
            ps_kt = psm.tile([128, 128], f32r, tag="tr")
            nc.tensor.transpose(ps_kt[:], Kc[:, ch], eyer[:])
            kctm = big.tile([128, 128], f32r, tag="kctm")
            nc.scalar.copy(kctm[:], ps_kt[:])
            ps_kt2 = psm.tile([128, 128], f32r, tag="tr")
            nc.tensor.transpose(ps_kt2[:], Ks[:, ch], eyer[:])
            kstm = big.tile([128, 128], f32r, tag="kstm")
            nc.scalar.copy(kstm[:], ps_kt2[:])
            ps_cc = pkv.tile([128, 2 * V], f32, tag="cc")
            nc.tensor.matmul(ps_cc[:, 0:V], kctm[:], gv[:],
                             start=True, stop=True)
            nc.tensor.matmul(ps_cc[:, V:2 * V], kstm[:], gv[:],
                             start=True, stop=True)
            if j == 0:
                nc.vector.tensor_copy(Ccos_sb[:], ps_cc[:, 0:V])
                nc.vector.tensor_copy(Csin_sb[:], ps_cc[:, V:2 * V])
            else:
                nc.vector.tensor_tensor(Ccos_sb[:], Ccos_sb[:],
                                        ps_cc[:, 0:V], Alu.add)
                nc.vector.tensor_tensor(Csin_sb[:], Csin_sb[:],
                                        ps_cc[:, V:2 * V], Alu.add)
            if "Cst" in debug:
                if "Cst" not in dbg_bufs:
                    dbg_bufs["Cst"] = dp("dbg_Cst", [ND * 128, NT], f32,
                                         isOutput=True)
                    dbg_shapes["Cst"] = True
                cdump = big.tile([128, V], f32, tag="rsc", name=f"cd{j}")
                nc.vector.tensor_copy(cdump[:], Ccos_sb[:].bitcast(f32))
                nc.sync.dma_start(out=dbg_bufs["Cst"][0:128, V * j:V * (j + 1)],
                                  in_=cdump[:])
                gdump = big.tile([128, V], f32, tag="rsc", name=f"gd{j}")
                nc.vector.tensor_copy(gdump[:], gv[:].bitcast(f32))
                nc.sync.dma_start(out=dbg_bufs["Cst"][128:256, V * j:V * (j + 1)],
                                  in_=gdump[:])

        def fill2b(pk):
            nc.vector.tensor_scalar(pk[:, 0:V], Ccos_sb[:], smask[:, 0:1],
                                    None, Alu.mult)
            nc.vector.tensor_scalar(pk[:, V:2 * V], Csin_sb[:],
                                    smask[:, 0:1], None, Alu.mult)
        rcv2b = exchange(2 * V, fill2b)
        rCcos = big.tile([128, V], f32r, tag="ccos")
        nc.vector.tensor_copy(rCcos[:], rcv2b[:, 0:V])
        rCsin = big.tile([128, V], f32r, tag="csin")
        nc.vector.tensor_copy(rCsin[:], rcv2b[:, V:2 * V])

        retr_fm = big.tile([V, NT], f32r, tag="gate")
        for j in range(NCH):
            ch = slice(128 * j, 128 * (j + 1))
            ps_r2 = pkv.tile([128, V], f32, tag="pr")
            nc.tensor.matmul(ps_r2[:], Qc[:, ch], rCcos[:],
                             start=True, stop=False)
            nc.tensor.matmul(ps_r2[:], Qs[:, ch], rCsin[:],
                             start=False, stop=True)
            t = big.tile([128, V], f32, tag="rsc")
            nc.vector.tensor_tensor(t[:], ps_r2[:],
                                    retr_sb[:, V * j:V * (j + 1)], Alu.add)
            nc.vector.tensor_scalar(t[:], t[:], rstd_g_tm[:, j:j + 1],
                                    None, Alu.mult)
            ps_f = psm.tile([V, 128], f32, tag="tr")
            nc.tensor.transpose(ps_f[:], t[:], eyef[:])
            nc.scalar.copy(retr_fm[:, ch], ps_f[:])
        dbg("retr_fm", retr_fm[:], 0)

        for blk in range(NBLK):
            cs = slice(TB * blk, TB * (blk + 1))
            for o in range(ND):
                ps = pb.tile([128, TB], f32, tag="lin")
                nc.tensor.matmul(ps[:], kvo_w[:, 128 * o:128 * (o + 1)],
                                 retr_fm[:, cs], start=True, stop=True)
                t = big.tile([128, TB], f32r, tag="xtm")
                nc.scalar.activation(t[:], ps[:], A.Identity,
                                     bias=bc("kvo_b", o))
                nc.sync.dma_start(out=d_pc[2][128 * o:128 * (o + 1), cs],
                                  in_=t[:])

        pkv.release()
        big.release()
        wleft.release()

        # ======== P8/P9: LN stats + o1 + o2 (fresh pool) ========
        p9 = tc.alloc_tile_pool(name="p9", bufs=1)
        pst = tc.alloc_tile_pool(name="pst", bufs=1, space="PSUM")

        pieces = []
        for pi in range(5):
            for d in range(ND):
                t = p9.tile([128, NT], f32r, tag=f"pc{pi}{d}")
                nc.sync.dma_start(out=t[:],
                                  in_=d_pc[pi][128 * d:128 * (d + 1), :])
                dbg(f"pc{pi}", t[:], d)
                pieces.append(t)

        m_row = p9.tile([1, NT], f32, tag="mrow")
        v_row = p9.tile([1, NT], f32, tag="vrow")
        ps_mean = pst.tile([1, NT], f32, tag="stat")
        for blk in range(NBLK):
            cs = slice(TB * blk, TB * (blk + 1))
            for i, pt in enumerate(pieces):
                nc.tensor.matmul(ps_mean[0:1, cs], onesr[:], pt[:, cs],
                                 start=(i == 0), stop=(i == len(pieces) - 1))
            nc.vector.tensor_scalar(m_row[:, cs], ps_mean[0:1, cs],
                                    1.0 / (5 * D), None, Alu.mult)
        ps_sq = pst.tile([1, NT], f32, tag="stat")
        for blk in range(NBLK):
            cs = slice(TB * blk, TB * (blk + 1))
            for i, pt in enumerate(pieces):
                sq = p9.tile([128, TB], f32r, tag="sqbuf")
                nc.scalar.activation(sq[:], pt[:, cs], A.Square)
                nc.tensor.matmul(ps_sq[0:1, cs], onesr[:], sq[:],
                                 start=(i == 0), stop=(i == len(pieces) - 1))
            msq = p9.tile([1, TB], f32, tag="msq")
            nc.vector.tensor_tensor(msq[:], m_row[0:1, cs], m_row[0:1, cs],
                                    Alu.mult)
            nc.vector.scalar_tensor_tensor(v_row[:, cs], ps_sq[0:1, cs],
                                           1.0 / (5 * D), msq[:],
                                           Alu.mult, Alu.subtract)
        dbg("ln_m", m_row[:], 0)
        dbg("ln_v", v_row[:], 0)
        rstd_row = p9.tile([1, NT], f32r, tag="rstdrow")
        nc.scalar.activation(rstd_row[:], v_row[:], A.Ln,
                             bias=bc("eps_ln", rows=1))
        nc.scalar.activation(rstd_row[:], rstd_row[:], A.Exp, scale=-0.5)
        mrstd_row = p9.tile([1, NT], f32r, tag="mrstdrow")
        nc.vector.tensor_tensor(mrstd_row[:], m_row[:], rstd_row[:], Alu.mult)
        rstd_bc = p9.tile([128, NT], f32, tag="rstdbc")
        mrstd_bc = p9.tile([128, NT], f32, tag="mrstdbc")
        for blk in range(NBLK):
            cs = slice(TB * blk, TB * (blk + 1))
            psb = psm.tile([128, TB], f32, tag="tr")
            nc.tensor.matmul(psb[:], ones_r1[:], rstd_row[0:1, cs],
                             start=True, stop=True)
            nc.scalar.copy(rstd_bc[:, cs], psb[:])
            psb2 = psm.tile([128, TB], f32, tag="tr")
            nc.tensor.matmul(psb2[:], ones_r1[:], mrstd_row[0:1, cs],
                             start=True, stop=True)
            nc.scalar.copy(mrstd_bc[:, cs], psb2[:])

        h1 = [p9.tile([128, NT], f32r, tag=f"h1{o}", name=f"h1{o}") for o in range(2 * ND)]
        for o in range(2 * ND):
            o1rows = []
            for i in range(5 * ND):
                t = p9.tile([128, 128], f32r, tag="wo1", bufs=6,
                            name=f"o1r{i}")
                nc.sync.dma_start(
                    out=t[:],
                    in_=wts["wT_o1"][128 * i:128 * (i + 1),
                                     128 * o:128 * (o + 1)])
                o1rows.append(t)
            for blk in range(NBLK):
                cs = slice(TB * blk, TB * (blk + 1))
                ps = pb.tile([128, TB], f32, tag="lin")
                for i, pt in enumerate(pieces):
                    nc.tensor.matmul(ps[:], o1rows[i][:], pt[:, cs],
                                     start=(i == 0),
                                     stop=(i == len(pieces) - 1))
                h1pre = p9.tile([128, TB], f32, tag="h1pre")
                nc.vector.tensor_tensor(h1pre[:], ps[:], rstd_bc[:, cs],
                                        Alu.mult)
                nc.vector.scalar_tensor_tensor(h1pre[:], mrstd_bc[:, cs],
                                               bc("negw", o), h1pre[:],
                                               Alu.mult, Alu.add)
                nc.scalar.activation(h1[o][:, cs], h1pre[:], A.Gelu,
                                     bias=bc("o1_b", o))
        for d in range(min(ND, 2 * ND)):
            dbg("h1", h1[d][:], d)

        o2rows = []
        for i in range(2 * ND):
            t = p9.tile([128, D], f32r, tag="wo2", bufs=8, name=f"o2r{i}")
            nc.sync.dma_start(out=t[:],
                              in_=wts["wT_o2"][128 * i:128 * (i + 1), :])
            o2rows.append(t)
        o2b_sb = p9.tile([1, D], f32r, tag="o2b")
        nc.sync.dma_start(out=o2b_sb[:], in_=wts["o2b_row"][:])
        for j in range(NCH):
            ch = slice(128 * j, 128 * (j + 1))
            ps = pb.tile([128, D], f32, tag="lin")
            for i in range(2 * ND):
                nc.tensor.matmul(ps[:], h1[i][:, ch], o2rows[i][:],
                                 start=(i == 0), stop=False)
            nc.tensor.matmul(ps[:], ones_r1[:], o2b_sb[:],
                             start=False, stop=True)
            xres = p9.tile([128, D], f32, tag="xres")
            nc.sync.dma_start(out=xres[:],
                              in_=x_ext[3 + 128 * j:3 + 128 * (j + 1), :])
            out_sb = p9.tile([128, D], f32, tag="outsb")
            nc.vector.tensor_tensor(out_sb[:], ps[:], xres[:], Alu.add)
            nc.sync.dma_start(out=y_out[128 * j:128 * (j + 1), :],
                              in_=out_sb[:])

        pst.release()
        p9.release()
        dram.release()
        psm.release()
        pb.release()
        con.release()

    fixup_excess_waits(nc)
    return nc, dbg_shapes


# ===================== host side =====================

def _prep_host(inputs):
    g = {k: np.asarray(v, dtype=np.float32) for k, v in inputs.items()}
    c = float(np.abs(g["mag_scale"]))
    absw = np.abs(g["omega_scale"])

    def pack4(wT, width):
        return np.ascontiguousarray(
            wT.reshape(ND, 128, width).transpose(1, 0, 2).reshape(
                128, ND * width))

    W = {}
    W["wT_tw"] = (g["tw_w"] * absw[:, None]).T
    W["wT_pi0"] = g["pi0_w"].T
    W["wT_pi2"] = g["pi2_w"].T
    W["wT_m1v"] = (g["m1v_w"] * c).T
    W["wT_mag"] = g["mag_w"].T
    W["wT_qo"] = g["qo_w"].T
    W["wT_cp"] = g["cp_w"].T
    W["wT_m1o"] = (g["m1o_w"] / math.sqrt(D)).T
    W["kepack"] = pack4(g["ke_w"].T, 128)
    W["vepack"] = pack4(g["ve_w"].T, V)
    W["sgpack"] = pack4(g["sg_w"].T, 1)
    W["wT_sk0"] = g["sk0_w"].T
    W["sk2pack"] = pack4(g["sk2_w"].T, 128)
    W["wT_kvo"] = (g["kvo_w"] / math.sqrt(P)).T
    o1w = g["o1_w"] * g["ln_g"][None, :]
    W["wT_o1"] = o1w.T
    W["wT_o2"] = g["o2_w"].T
    W["negWsum"] = -o1w.sum(axis=1)[None, :]
    W["o2b_row"] = g["o2_b"][None, :]
    W["ones_col"] = np.ones((128, 1), np.float32)
    W["ones_row1"] = np.ones((1, 128), np.float32)
    W["eye_r"] = np.eye(128, dtype=np.float32)
    W = {k: np.ascontiguousarray(v, dtype=np.float32) for k, v in W.items()}

    b1p = g["o1_b"] + g["o1_w"] @ g["ln_b"]

    bias = np.zeros((128, NBIAS), np.float32)
    def put(name, vec, i=0):
        v = np.asarray(vec, np.float32).ravel()
        bias[:len(v), BC[name] + i] = v
    for d in range(ND):
        sl = slice(128 * d, 128 * (d + 1))
        put("tw_b", (g["tw_b"] * absw)[sl], d)
        put("pi0_b", g["pi0_b"][sl], d)
        put("pi2_b", g["pi2_b"][sl], d)
        put("m1v_b", (g["m1v_b"] * c)[sl], d)
        put("mag_b", (0.5 * g["mag_b"])[sl], d)
        put("qo_b", g["qo_b"][sl], d)
        put("cp_b", g["cp_b"][sl], d)
        put("m1o_b", g["m1o_b"][sl], d)
        put("sk0_b", g["sk0_b"][sl], d)
        put("kvo_b", g["kvo_b"][sl], d)
        put("lc_b", g["lc_b"][sl], d)
        put("cg_b", (0.5 * g["cg_b"])[sl], d)
        for k in range(K):
            put("lc_w", g["lc_w"][sl, 0, k], 4 * d + k)
            put("cg_w", g["cg_w"][sl, 0, k], 4 * d + k)
    put("ke_b", g["ke_b"])
    put("ve_b", g["ve_b"])
    put("sg_b", 0.5 * g["sg_b"])
    put("sk2_b", g["sk2_b"])
    for o in range(8):
        put("o1_b", b1p[128 * o:128 * (o + 1)], o)
        put("negw", W["negWsum"][0, 128 * o:128 * (o + 1)], o)
    put("halfpi", np.full(128, HALF_PI))
    put("eps_mag", np.full(128, 1e-8))
    put("c_mag", np.full(128, c))
    put("eps_ln", np.full(128, 1e-5))

    tril = np.triu(np.ones((128, 128), np.float32))
    eye = np.eye(128, dtype=np.float32)
    pos = np.arange(1, L + 1, dtype=np.float32)

    x = g["x"]
    in_maps = []
    for core in range(N_CORES):
        b, h = core // 2, core % 2
        xe = np.zeros((NT + 3, D), np.float32)
        if h == 0:
            xe[3:] = x[b, 0:NT]
        else:
            xe[:] = x[b, NT - 3:2 * NT]
        rp = np.broadcast_to(1.0 / pos[h * NT:(h + 1) * NT][None, :],
                             (128, NT)).copy()
        m = {"x_ext": xe, "bias_pack": bias, "recip_pos": rp,
             "eye_f": eye, "tril": tril,
             "send_mask": np.full((128, 1), 1.0 - h, np.float32),
             "use_mask": np.full((128, 1), float(h), np.float32)}
        m.update(W)
        in_maps.append(m)
    return in_maps


_CACHE = {}

def _get_built(debug=()):
    key = tuple(sorted(debug))
    if key not in _CACHE:
        _CACHE[key] = build_nc(key)
    return _CACHE[key]


def run_cores(inputs, debug=(), trace=False):
    from concourse.bass_utils import run_bass_kernel_spmd
    nc, dbg_shapes = _get_built(debug)
    in_maps = _prep_host(inputs)
    res = run_bass_kernel_spmd(nc, in_maps, list(range(N_CORES)),
                               trace=trace)
    return res


def kernel(**inputs):
    results = run_cores(inputs).results
    out = np.empty((B, L, D), np.float32)
    for core in range(N_CORES):
        b, h = core // 2, core % 2
        out[b, h * NT:(h + 1) * NT] = results[core]["y"]
    return out



# revision 23
# speedup vs baseline: 1.0403x; 1.0403x over previous
"""Trainium2 Bass kernel for nn_EvolvingLocalConvBlock (v2, bf16-resident).

Sharding: 8 cores = 4 samples x 2 sequence halves (1024 tokens each).
Cross-core cumsum carries via three pairwise AllReduces (even core sends
masked totals; odd core consumes).

v2 design vs v1:
- No DRAM staging: all [128, NT] intermediates live in SBUF in bf16
  (scans keep f32 internal state; phases assembled in f32 scratch).
- x uploaded pre-transposed (feature-major) in bf16: no PE transposes.
- All weights bf16 (halves DMA); matmuls bf16 (full PE rate).
- Elementwise split across DVE (nc.vector) and Pool (nc.gpsimd).
- Collectives ordered so ~30us of independent work hides each AllReduce.
- Activations grouped by table set: A(gelu/tanh) -> B(ln/exp) ->
  C(sin) -> B -> A.
"""
import sys
sys.path.insert(0, '/opt/trn_rl_repo')

import math
import numpy as np

import concourse.bass as bass
import concourse.mybir as mybir
from concourse.tile import TileContext

B, L, D, P, V, K = 4, 2048, 512, 128, 8, 4
N_CORES = 8
NT = L // 2
NCH = NT // 128
ND = D // 128
NBLK = 2
TB = NT // NBLK

f32 = mybir.dt.float32
bf16 = mybir.dt.bfloat16
A = mybir.ActivationFunctionType
Alu = mybir.AluOpType

TWO_PI = 2.0 * math.pi
HALF_PI = math.pi / 2.0

# ---- bias_pack column map ----
BC = {}
_ncols = 0
def _bc(name, n):
    global _ncols
    BC[name] = _ncols
    _ncols += n
for _n, _k in [("tw_b", ND), ("pi0_b", ND), ("pi2_b", ND), ("m1v_b", ND),
               ("mag_b", ND), ("qo_b", ND), ("cp_b", ND), ("m1o_b", ND),
               ("ke_b", 1), ("ve_b", 1), ("sg_b", 1), ("sk0_b", ND),
               ("sk2_b", 1), ("kvo_b", ND), ("o1_b", 8), ("negw", 8),
               ("lc_b", ND), ("cg_b", ND),
               ("lc_w", ND * K), ("cg_w", ND * K),
               ("halfpi", 1), ("eps_mag", 1), ("c_mag", 1), ("eps_ln", 1)]:
    _bc(_n, _k)
NBIAS = _ncols


def fixup_excess_waits(nc, max_waits=1, max_updates=1):
    """This walrus accepts at most one sync wait/update per instruction;
    hoist extras onto adjacent same-engine NoOps."""
    for f in nc.m.functions:
        for bb in f.blocks:
            new = []
            changed = False
            for ins in bb.instructions:
                si = getattr(ins, 'sync_info', None)
                if si is None:
                    new.append(ins)
                    continue
                w = list(si.on_wait) if si.on_wait else []
                if len(w) > max_waits:
                    excess, keep = w[:-max_waits], w[-max_waits:]
                    for i in range(0, len(excess), max_waits):
                        nop = mybir.InstNoOp(name=f"{ins.name}-hw{i}",
                                             engine=ins.engine, ins=[], outs=[])
                        nop.sync_info = mybir.SyncInfo(
                            on_wait=excess[i:i + max_waits], on_update=[])
                        new.append(nop)
                    si.on_wait = keep
                    changed = True
                new.append(ins)
                u = list(si.on_update) if si.on_update else []
                if len(u) > max_updates:
                    excess_u, keep_u = u[max_updates:], u[:max_updates]
                    for i in range(0, len(excess_u), max_updates):
                        nop = mybir.InstNoOp(name=f"{ins.name}-hu{i}",
                                             engine=ins.engine, ins=[], outs=[])
                        nop.sync_info = mybir.SyncInfo(
                            on_wait=[], on_update=excess_u[i:i + max_updates])
                        new.append(nop)
                    si.on_update = keep_u
                    changed = True
            if changed:
                bb.instructions = new


def build_nc(debug=()):
    import concourse.tile_utils as tile_utils
    tile_utils.max_sbuf_usage = 206 * 1024

    nc = bass.Bass()
    dp = nc.declare_dram_parameter

    xT_in = dp("xT", [D, NT + 3], bf16, isOutput=False)
    xres_in = dp("x_res", [NT, D], f32, isOutput=False)
    y_out = dp("y", [NT, D], f32, isOutput=True)

    wts = {}
    for name, shape in [
        ("wT_tw", [D, D]), ("wT_pi0", [D, D]), ("wT_pi2", [D, D]),
        ("wT_m1v", [D, D]), ("wT_mag", [D, D]), ("wT_qo", [D, D]),
        ("wT_cp", [D, D]), ("wT_m1o", [D, D]),
        ("kepack", [128, ND * 128]), ("vepack", [128, ND * V]),
        ("sgpack", [128, ND]), ("wT_sk0", [2 * D, D]),
        ("sk2pack", [128, ND * 128]), ("wT_kvo", [V, D]),
        ("wT_o1", [5 * D, 2 * D]), ("wT_o2", [2 * D, D]),
        ("o2b_row", [1, D]),
        ("ones_col", [128, 1]), ("ones_row1", [1, 128]),
        ("eye_b", [128, 128]), ("tril_b", [128, 128]),
        ("recip_pos", [128, NT]),
    ]:
        wts[name] = dp(name, shape, bf16, isOutput=False)
    eyef_in = dp("eye_f", [V, V], f32, isOutput=False)
    bias_in = dp("bias_pack", [128, NBIAS], f32, isOutput=False)
    smask_in = dp("send_mask", [128, 1], f32, isOutput=False)
    umask_in = dp("use_mask", [128, 1], f32, isOutput=False)

    dbg_shapes = {}
    RG = [[0, 1], [2, 3], [4, 5], [6, 7]]

    with TileContext(nc) as tc:
        con = tc.alloc_tile_pool(name="con", bufs=1, side="left")
        keep = tc.alloc_tile_pool(name="keep", bufs=1, side="left")
        work = tc.alloc_tile_pool(name="work", bufs=1)
        pb = tc.alloc_tile_pool(name="pb", bufs=4, space="PSUM")
        psm = tc.alloc_tile_pool(name="psm", bufs=1, space="PSUM")
        pkv = tc.alloc_tile_pool(name="pkv", bufs=1, space="PSUM")
        dram = tc.alloc_tile_pool(name="dram", bufs=1, space="DRAM")

        dbg_bufs = {}
        def dbg(name, ap, part, pool=None):
            """Dump (rows, NT) AP into 128-row slot `part` of a debug out."""
            if name not in debug:
                return
            pool = pool or work
            r = ap.shape[0]
            if name not in dbg_bufs:
                dbg_bufs[name] = dp("dbg_" + name, [ND * 128, NT], f32,
                                    isOutput=True)
                dbg_shapes[name] = True
            t = dbg_bufs[name]
            if ap.dtype != f32:
                cv = pool.tile([r, ap.shape[1]], f32, tag="dbgcv",
                               name=f"dbgcv{name}{part}")
                nc.vector.tensor_copy(cv[:], ap)
                ap = cv[:]
            nc.sync.dma_start(out=t[128 * part:128 * part + r, :], in_=ap)

        # ---------------- constants ----------------
        bias = con.tile([128, NBIAS], f32, tag="bias")
        nc.sync.dma_start(out=bias[:], in_=bias_in[:])
        def bc(name, i=0, rows=128):
            return bias[0:rows, BC[name] + i:BC[name] + i + 1]
        eyeb = con.tile([128, 128], bf16, tag="eyeb")
        nc.sync.dma_start(out=eyeb[:], in_=wts["eye_b"][:])
        trilb = con.tile([128, 128], bf16, tag="trilb")
        nc.sync.dma_start(out=trilb[:], in_=wts["tril_b"][:])
        eyef = con.tile([V, V], f32, tag="eyef")
        nc.sync.dma_start(out=eyef[:], in_=eyef_in[:])
        smask = con.tile([128, 1], f32, tag="smask")
        nc.sync.dma_start(out=smask[:], in_=smask_in[:])
        umask = con.tile([128, 1], f32, tag="umask")
        nc.sync.dma_start(out=umask[:], in_=umask_in[:])
        onesb = con.tile([128, 1], bf16, tag="onesb")
        nc.sync.dma_start(out=onesb[:], in_=wts["ones_col"][:])
        ones_r1 = con.tile([1, 128], bf16, tag="onesr1")
        nc.sync.dma_start(out=ones_r1[:], in_=wts["ones_row1"][:])
        recip = con.tile([128, NT], bf16, tag="recip")
        nc.sync.dma_start(out=recip[:], in_=wts["recip_pos"][:])
        zerosb = con.tile([128, NT], bf16, tag="zerosb")
        nc.vector.memset(zerosb[:], 0.0)
        lastc = con.tile([128, 16], f32, tag="lastc")

        # pieces (live to the end)
        pc = [[keep.tile([128, NT], bf16, tag=f"pc{p}{d}", name=f"pc{p}{d}")
               for d in range(ND)] for p in range(5)]

        # ---------------- P0: x load (already feature-major) ----------
        x_fm = []
        for d in range(ND):
            t = work.tile([128, NT + 3], bf16, tag=f"x{d}", name=f"x{d}")
            nc.sync.dma_start(out=t[:],
                              in_=xT_in[128 * d:128 * (d + 1), :])
            x_fm.append(t)
        xin = [t[:, 3:3 + NT] for t in x_fm]

        # long-lived bf16 stores
        som = [work.tile([128, NT], bf16, tag=f"som{d}", name=f"som{d}")
               for d in range(ND)]
        Ssig = [work.tile([128, NT], bf16, tag=f"ssg{d}", name=f"ssg{d}")
                for d in range(ND)]
        Sx = [work.tile([128, NT], bf16, tag=f"sx{d}", name=f"sx{d}")
              for d in range(ND)]
        # f32 scratch (rotating)
        def scr(tag):
            return work.tile([128, NT], f32, tag=tag, name=tag)

        # ---------------- helpers ----------------
        def load_wrows(name, nin, nout, tag="w4", bufs=4):
            rows = []
            for i in range(nin):
                t = work.tile([128, nout], bf16, tag=tag, bufs=bufs,
                              name=f"{name}r{i}")
                nc.sync.dma_start(out=t[:],
                                  in_=wts[name][128 * i:128 * (i + 1), :])
                rows.append(t)
            return rows

        def mm_big(wname, rhs_tiles, epilogue, nout=D, tag="w4", bufs=4):
            rows = load_wrows(wname, len(rhs_tiles), nout, tag=tag, bufs=bufs)
            for o in range(nout // 128):
                for blk in range(NBLK):
                    cs = slice(TB * blk, TB * (blk + 1))
                    ps = pb.tile([128, TB], f32, tag="lin")
                    for i, r in enumerate(rhs_tiles):
                        nc.tensor.matmul(ps[:],
                                         rows[i][:, 128 * o:128 * (o + 1)],
                                         r[:, cs], start=(i == 0),
                                         stop=(i == len(rhs_tiles) - 1))
                    epilogue(o, blk, ps)

        def mm_packed(wname, rhs_tiles, out_rows, epilogue):
            nin = len(rhs_tiles)
            wrow = work.tile([128, nin * out_rows], bf16, tag="wp1",
                             bufs=2, name=wname)
            nc.sync.dma_start(out=wrow[:], in_=wts[wname][:])
            for blk in range(NBLK):
                cs = slice(TB * blk, TB * (blk + 1))
                ps = pb.tile([out_rows, TB], f32, tag="lin")
                for i in range(nin):
                    nc.tensor.matmul(ps[:],
                                     wrow[:, out_rows * i:out_rows * (i + 1)],
                                     rhs_tiles[i][:, cs],
                                     start=(i == 0), stop=(i == nin - 1))
                epilogue(blk, ps)

        def scan_dve(dst_ap, src_ap, rows=128):
            nc.vector.tensor_tensor_scan(dst_ap, zerosb[0:rows, 0:NT],
                                         src_ap, 0.0, Alu.add, Alu.add)

        def exchange(n, fill):
            pk = work.tile([128, n], f32, tag="pk", name=f"pk{n}")
            nc.vector.memset(pk[:], 0.0)
            fill(pk)
            cin = dram.tile([128, n], f32, tag=f"ci{n}")
            cout = dram.tile([128, n], f32, tag=f"co{n}")
            nc.sync.dma_start(out=cin[:], in_=pk[:])
            nc.gpsimd.collective_compute(
                "AllReduce", Alu.add, replica_groups=RG,
                ins=[cin.opt()], outs=[cout.opt()])
            rcv = work.tile([128, n], f32, tag=f"rc{n}", name=f"rc{n}")
            nc.sync.dma_start(out=rcv[:], in_=cout[:])
            rcvu = work.tile([128, n], f32, tag=f"ru{n}", name=f"ru{n}")
            nc.vector.tensor_scalar(rcvu[:], rcv[:], umask[:, 0:1], None,
                                    Alu.mult)
            return rcvu

        # ======== P1a: tw/mag/sg -> scans -> exchange1 ========
        # S_x scans first (no deps beyond x) on Pool
        for d in range(ND):
            scan_dve(Sx[d][:], xin[d])
            nc.vector.tensor_copy(lastc[:, 8 + d:9 + d], Sx[d][:, NT - 1:NT])

        twrows = load_wrows("wT_tw", ND, D)
        for o in range(ND):
            omt = scr("sB")
            for blk in range(NBLK):
                cs = slice(TB * blk, TB * (blk + 1))
                ps = pb.tile([128, TB], f32, tag="lin")
                for i in range(ND):
                    nc.tensor.matmul(ps[:], twrows[i][:, 128 * o:128 * (o + 1)],
                                     xin[i][:, cs], start=(i == 0),
                                     stop=(i == ND - 1))
                nc.scalar.activation(omt[:, cs], ps[:], A.Identity,
                                     bias=bc("tw_b", o))
            scan_dve(som[o][:], omt[:])
            nc.vector.tensor_copy(lastc[:, o:o + 1], som[o][:, NT - 1:NT])

        sig = [work.tile([128, NT], bf16, tag=f"sig{d}", name=f"sig{d}")
               for d in range(ND)]
        def ep_sig(o, blk, ps):
            ap = sig[o][:, TB * blk:TB * (blk + 1)]
            nc.scalar.activation(ap, ps[:], A.Tanh, bias=bc("mag_b", o),
                                 scale=0.5)
            nc.gpsimd.tensor_scalar(ap, ap, 0.5, 0.5, Alu.mult, Alu.add)
        mm_big("wT_mag", xin, ep_sig)
        for d in range(ND):
            scan_dve(Ssig[d][:], sig[d][:])
            nc.vector.tensor_copy(lastc[:, 4 + d:5 + d],
                                  Ssig[d][:, NT - 1:NT])
            dbg("sig", sig[d][:], d)

        gate = work.tile([1, NT], f32, tag="gate", name="gate")
        def ep_sg(blk, ps):
            ap = gate[:, TB * blk:TB * (blk + 1)]
            nc.scalar.activation(ap, ps[:], A.Tanh, bias=bc("sg_b", rows=1),
                                 scale=0.5)
            nc.gpsimd.tensor_scalar(ap, ap, 0.5, 0.5, Alu.mult, Alu.add)
        mm_packed("sgpack", xin, 1, ep_sg)
        S_gate = work.tile([1, NT], f32, tag="sgate", name="sgate")
        scan_dve(S_gate[:], gate[:], rows=1)

        def fill1(pk):
            for c in range(12):
                nc.vector.tensor_scalar(pk[:, c:c + 1], lastc[:, c:c + 1],
                                        smask[:, 0:1], None, Alu.mult)
            nc.vector.tensor_scalar(pk[0:1, 12:13], S_gate[:, NT - 1:NT],
                                    smask[0:1, 0:1], None, Alu.mult)
        rcv1 = exchange(13, fill1)

        # ======== P1b (exchange1 in flight) ========
        g0 = [work.tile([128, NT], bf16, tag=f"g0{d}", name=f"g0{d}")
              for d in range(ND)]
        def ep_g0(o, blk, ps):
            nc.scalar.activation(g0[o][:, TB * blk:TB * (blk + 1)], ps[:],
                                 A.Gelu, bias=bc("pi0_b", o))
        mm_big("wT_pi0", xin, ep_g0)

        v1 = [work.tile([128, NT], bf16, tag=f"v1{d}", name=f"v1{d}")
              for d in range(ND)]
        def ep_v1(o, blk, ps):
            nc.scalar.activation(v1[o][:, TB * blk:TB * (blk + 1)], ps[:],
                                 A.Identity, bias=bc("m1v_b", o))
        mm_big("wT_m1v", xin, ep_v1)

        t_ke = work.tile([128, NT], bf16, tag="tke", name="tke")
        def ep_ke(blk, ps):
            nc.scalar.activation(t_ke[:, TB * blk:TB * (blk + 1)], ps[:],
                                 A.Tanh, bias=bc("ke_b"))
        mm_packed("kepack", xin, 128, ep_ke)

        vals = work.tile([V, NT], f32, tag="vals", name="vals")
        def ep_ve(blk, ps):
            nc.scalar.activation(vals[:, TB * blk:TB * (blk + 1)], ps[:],
                                 A.Identity, bias=bc("ve_b", rows=V))
        mm_packed("vepack", xin, V, ep_ve)

        # conv branch: co chain on DVE (stt), cg chain on Pool (ts+tt)
        convg = []
        for d in range(ND):
            co = scr("sB")
            nc.vector.tensor_scalar(co[:], x_fm[d][:, 0:NT],
                                    bc("lc_w", 4 * d + 0), bc("lc_b", d),
                                    Alu.mult, Alu.add)
            for k in range(1, K):
                nc.vector.scalar_tensor_tensor(
                    co[:], x_fm[d][:, k:k + NT], bc("lc_w", 4 * d + k), co[:],
                    Alu.mult, Alu.add)
            cg = scr("sC")
            nc.gpsimd.tensor_scalar(cg[:], x_fm[d][:, 0:NT],
                                    bc("cg_w", 4 * d + 0), bc("cg_b", d),
                                    Alu.mult, Alu.add)
            for k in range(1, K):
                pk_t = scr("sD")
                nc.gpsimd.tensor_scalar(pk_t[:], x_fm[d][:, k:k + NT],
                                        bc("cg_w", 4 * d + k), None, Alu.mult)
                nc.gpsimd.tensor_tensor(cg[:], cg[:], pk_t[:], Alu.add)
            nc.scalar.activation(cg[:], cg[:], A.Tanh, scale=0.5)
            nc.gpsimd.tensor_scalar(cg[:], cg[:], 0.5, 0.5,
                                    Alu.mult, Alu.add)
            gt = work.tile([128, NT], bf16, tag=f"cvg{d}", name=f"cvg{d}")
            nc.gpsimd.tensor_tensor(gt[:], cg[:], co[:], Alu.mult)
            convg.append(gt)
            dbg("convg", gt[:], d)

        def ep_store(dst_list, bname):
            def ep(o, blk, ps):
                nc.scalar.activation(
                    dst_list[o][:, TB * blk:TB * (blk + 1)], ps[:],
                    A.Identity, bias=bc(bname, o))
            return ep
        mm_big("wT_cp", [t[:] for t in convg], ep_store(pc[0], "cp_b"))

        pi2o = [work.tile([128, NT], bf16, tag=f"pio{d}", name=f"pio{d}")
                for d in range(ND)]
        def ep_pi2(o, blk, ps):
            nc.scalar.activation(pi2o[o][:, TB * blk:TB * (blk + 1)], ps[:],
                                 A.Identity, bias=bc("pi2_b", o))
        mm_big("wT_pi2", [t[:] for t in g0], ep_pi2)

        qoo = [work.tile([128, NT], bf16, tag=f"qoo{d}", name=f"qoo{d}")
               for d in range(ND)]
        def ep_qo(o, blk, ps):
            nc.scalar.activation(qoo[o][:, TB * blk:TB * (blk + 1)], ps[:],
                                 A.Identity, bias=bc("qo_b", o))
        mm_big("wT_qo", xin, ep_qo)

        # ======== P3.1 (needs rcv1) [table set A] ========
        sk0rows = load_wrows("wT_sk0", 2 * ND, D, tag="w8", bufs=8)
        gsk = [work.tile([128, NT], bf16, tag=f"gsk{d}", name=f"gsk{d}")
               for d in range(ND)]
        for blk in range(NBLK):
            cs = slice(TB * blk, TB * (blk + 1))
            ctxc = []
            for d in range(ND):
                cf = scr("sB")
                nc.vector.tensor_scalar(cf[:, cs], Sx[d][:, cs],
                                        rcv1[:, 8 + d:9 + d], None, Alu.add)
                t = work.tile([128, TB], bf16, tag=f"ctx{d}",
                              name=f"ctx{d}_{blk}")
                nc.gpsimd.tensor_tensor(t[:], cf[:, cs], recip[:, cs],
                                        Alu.mult)
                ctxc.append(t)
            for o in range(ND):
                ps = pb.tile([128, TB], f32, tag="lin")
                for i in range(ND):
                    nc.tensor.matmul(ps[:],
                                     sk0rows[i][:, 128 * o:128 * (o + 1)],
                                     xin[i][:, cs], start=(i == 0),
                                     stop=False)
                for i in range(ND):
                    nc.tensor.matmul(
                        ps[:], sk0rows[ND + i][:, 128 * o:128 * (o + 1)],
                        ctxc[i][:], start=False, stop=(i == ND - 1))
                nc.scalar.activation(gsk[o][:, cs], ps[:], A.Gelu,
                                     bias=bc("sk0_b", o))
        t_sk = work.tile([128, NT], bf16, tag="tsk", name="tsk")
        def ep_sk2(blk, ps):
            nc.scalar.activation(t_sk[:, TB * blk:TB * (blk + 1)], ps[:],
                                 A.Tanh, bias=bc("sk2_b"))
        mm_packed("sk2pack", [t[:] for t in gsk], 128, ep_sk2)

        # ======== P3.2 [table set B: ln/exp] ========
        rstdm = []
        for d in range(ND):
            rf = scr("sB")
            nc.vector.tensor_scalar(rf[:], Ssig[d][:], rcv1[:, 4 + d:5 + d],
                                    None, Alu.add)
            nc.scalar.activation(rf[:], rf[:], A.Ln, bias=bc("eps_mag"),
                                 scale=bc("c_mag"))
            t = work.tile([128, NT], bf16, tag=f"rsm{d}", name=f"rsm{d}")
            nc.scalar.activation(t[:], rf[:], A.Exp, scale=-0.5)
            rstdm.append(t)
            dbg("rstdm", t[:], d)

        gn_row = S_gate
        nc.vector.tensor_scalar(gn_row[:], S_gate[:], rcv1[0:1, 12:13],
                                None, Alu.add)
        nc.vector.tensor_scalar(gn_row[:], gn_row[:], 1.0, None, Alu.max)
        nc.scalar.activation(gn_row[:], gn_row[:], A.Ln)
        nc.scalar.activation(gn_row[:], gn_row[:], A.Exp, scale=-0.5)
        dbg("gnr", gn_row[:], 0)
        rstd_g_tm = work.tile([128, NCH], f32, tag="rgtm", name="rgtm")
        for j in range(NCH):
            psj = psm.tile([128, 1], f32, tag="tr")
            nc.tensor.transpose(psj[:], gn_row[0:1, 128 * j:128 * (j + 1)],
                                eyef[0:1, 0:1])
            nc.scalar.copy(rstd_g_tm[:, j:j + 1], psj[:])

        # ======== P3.3 [table set C: sin] ========
        def phase_cs(tin, cname, sname):
            s_t = work.tile([128, NT], bf16, tag=sname, name=sname)
            nc.scalar.activation(s_t[:], tin[:], A.Sin, scale=math.pi)
            m = scr("sB")
            nc.vector.tensor_scalar(m[:], tin[:], 0.5, None, Alu.is_gt)
            nc.vector.scalar_tensor_tensor(m[:], m[:], -2.0, tin[:],
                                           Alu.mult, Alu.add)
            c_t = work.tile([128, NT], bf16, tag=cname, name=cname)
            nc.scalar.activation(c_t[:], m[:], A.Sin, scale=math.pi,
                                 bias=bc("halfpi"))
            return c_t, s_t
        Qc, Qs = phase_cs(t_ke, "Qc", "Qs")
        Kc, Ks = phase_cs(t_sk, "Kc", "Ks")
        dbg("Qc", Qc[:], 0)
        dbg("Kc", Kc[:], 0)

        cosq, sinq, posret = [], [], []
        for o in range(ND):
            phic = scr("sA")
            nc.vector.scalar_tensor_tensor(phic[:], som[o][:],
                                           rcv1[:, o:o + 1], pi2o[o][:],
                                           Alu.add, Alu.add)
            dbg("phi", phic[:], o)
            cphi = scr("sB")
            nc.scalar.activation(cphi[:], phic[:], A.Sin, bias=bc("halfpi"))
            sphi = scr("sC")
            nc.scalar.activation(sphi[:], phic[:], A.Sin)
            nc.gpsimd.tensor_tensor(pc[3][o][:], xin[o], cphi[:], Alu.mult)
            nc.vector.tensor_tensor(pc[4][o][:], xin[o], sphi[:], Alu.mult)
            wv = scr("sD")
            nc.gpsimd.tensor_tensor(wv[:], sig[o][:], v1[o][:], Alu.mult)
            nc.vector.tensor_tensor(cphi[:], wv[:], cphi[:], Alu.mult)
            scan_dve(Ssig[o][:], cphi[:])      # Ssig becomes Sc
            nc.vector.tensor_copy(lastc[:, 4 + o:5 + o],
                                  Ssig[o][:, NT - 1:NT])
            dbg("Sc", Ssig[o][:], o)
            nc.gpsimd.tensor_tensor(sphi[:], wv[:], sphi[:], Alu.mult)
            scan_dve(Sx[o][:], sphi[:])       # Sx becomes Ss
            nc.vector.tensor_copy(lastc[:, 8 + o:9 + o],
                                  Sx[o][:, NT - 1:NT])
            # phi_q = phi + qo_out (qo bias already applied)
            nc.vector.tensor_tensor(phic[:], phic[:], qoo[o][:], Alu.add)
            sq_t = v1[o]                        # reuse v1 tile for sinq
            nc.scalar.activation(sq_t[:], phic[:], A.Sin)
            sinq.append(sq_t)
            m = wv
            nc.vector.tensor_scalar(m[:], phic[:], HALF_PI, None, Alu.is_gt)
            nc.vector.scalar_tensor_tensor(phic[:], m[:], -TWO_PI, phic[:],
                                           Alu.mult, Alu.add)
            cq_t = g0[o]                        # reuse g0 tile for cosq
            nc.scalar.activation(cq_t[:], phic[:], A.Sin, bias=bc("halfpi"))
            cosq.append(cq_t)
            dbg("cosq", cq_t[:], o)

        def fill2a(pk):
            for c in range(8):
                nc.vector.tensor_scalar(pk[:, c:c + 1], lastc[:, 4 + c:5 + c],
                                        smask[:, 0:1], None, Alu.mult)
        rcv2a = exchange(8, fill2a)

        # ======== P3.4 (exchange2a in flight): kv chunk loop ========
        retr_sb = work.tile([128, V * NCH], f32, tag="retr", name="retr")
        Cacc_c = work.tile([128, V], f32, tag="caccc", name="caccc")
        Cacc_s = work.tile([128, V], f32, tag="caccs", name="caccs")
        Cbf_c = work.tile([128, V], bf16, tag="cbfc", name="cbfc")
        Cbf_s = work.tile([128, V], bf16, tag="cbfs", name="cbfs")
        for j in range(NCH):
            ch = slice(128 * j, 128 * (j + 1))
            ps_st = psm.tile([128, 128], f32, tag="tr")
            nc.tensor.matmul(ps_st[:], Kc[:, ch], Qc[:, ch],
                             start=True, stop=False)
            nc.tensor.matmul(ps_st[:], Ks[:, ch], Qs[:, ch],
                             start=False, stop=True)
            st_sb = work.tile([128, 128], bf16, tag="stsb", name=f"st{j}")
            nc.vector.tensor_tensor(st_sb[:], ps_st[:], trilb[:], Alu.mult)
            ps_v = psm.tile([128, V + 1], f32, tag="tr")
            nc.tensor.transpose(ps_v[:, 0:V], vals[:, ch], eyef[0:V, 0:V])
            nc.tensor.transpose(ps_v[:, V:V + 1], gate[0:1, ch],
                                eyef[0:1, 0:1])
            gv = work.tile([128, V], bf16, tag="gv", name=f"gv{j}")
            nc.vector.tensor_scalar(gv[:], ps_v[:, 0:V], ps_v[:, V:V + 1],
                                    None, Alu.mult)
            ps_r = pkv.tile([128, V], f32, tag="pr")
            nc.tensor.matmul(ps_r[:], st_sb[:], gv[:], start=True,
                             stop=(j == 0))
            if j > 0:
                nc.tensor.matmul(ps_r[:], Qc[:, ch], Cbf_c[:],
                                 start=False, stop=False)
                nc.tensor.matmul(ps_r[:], Qs[:, ch], Cbf_s[:],
                                 start=False, stop=True)
            nc.scalar.copy(retr_sb[:, V * j:V * (j + 1)], ps_r[:])
            ps_kt = psm.tile([128, 128], bf16, tag="tr2")
            nc.tensor.transpose(ps_kt[:], Kc[:, ch], eyeb[:])
            kctm = work.tile([128, 128], bf16, tag="kctm", name=f"kc{j}")
            nc.scalar.copy(kctm[:], ps_kt[:])
            ps_kt2 = psm.tile([128, 128], bf16, tag="tr2")
            nc.tensor.transpose(ps_kt2[:], Ks[:, ch], eyeb[:])
            kstm = work.tile([128, 128], bf16, tag="kstm", name=f"ks{j}")
            nc.scalar.copy(kstm[:], ps_kt2[:])
            ps_cc = pkv.tile([128, 2 * V], f32, tag="cc")
            nc.tensor.matmul(ps_cc[:, 0:V], kctm[:], gv[:],
                             start=True, stop=True)
            nc.tensor.matmul(ps_cc[:, V:2 * V], kstm[:], gv[:],
                             start=True, stop=True)
            if j == 0:
                nc.vector.tensor_copy(Cacc_c[:], ps_cc[:, 0:V])
                nc.vector.tensor_copy(Cacc_s[:], ps_cc[:, V:2 * V])
            else:
                nc.vector.tensor_tensor(Cacc_c[:], Cacc_c[:],
                                        ps_cc[:, 0:V], Alu.add)
                nc.vector.tensor_tensor(Cacc_s[:], Cacc_s[:],
                                        ps_cc[:, V:2 * V], Alu.add)
            nc.gpsimd.tensor_copy(Cbf_c[:], Cacc_c[:])
            nc.gpsimd.tensor_copy(Cbf_s[:], Cacc_s[:])

        def fill2b(pk):
            nc.vector.tensor_scalar(pk[:, 0:V], Cacc_c[:], smask[:, 0:1],
                                    None, Alu.mult)
            nc.vector.tensor_scalar(pk[:, V:2 * V], Cacc_s[:],
                                    smask[:, 0:1], None, Alu.mult)
        rcv2b = exchange(2 * V, fill2b)

        # ======== P3.5 (exchange2b in flight): mem1 + m1o ========
        for d in range(ND):
            t1 = scr("sA")
            nc.vector.scalar_tensor_tensor(t1[:], Ssig[d][:],
                                           rcv2a[:, d:d + 1],
                                           cosq[d][:], Alu.add, Alu.mult)
            t2 = scr("sB")
            nc.gpsimd.tensor_scalar(t2[:], Sx[d][:], rcv2a[:, 4 + d:5 + d],
                                    None, Alu.add)
            nc.gpsimd.tensor_tensor(t2[:], t2[:], sinq[d][:], Alu.mult)
            nc.vector.tensor_tensor(t1[:], t1[:], t2[:], Alu.add)
            pr = sig[d]                         # reuse sig tile for pos_ret
            nc.gpsimd.tensor_tensor(pr[:], t1[:], rstdm[d][:], Alu.mult)
            posret.append(pr)
            dbg("pos_ret", pr[:], d)
        mm_big("wT_m1o", [t[:] for t in posret], ep_store(pc[1], "m1o_b"))

        # ======== P3.6 (needs rcv2b): kv retrieval + kvo ========
        rCc = work.tile([128, V], bf16, tag="cbfc2", name="rCc")
        nc.vector.tensor_copy(rCc[:], rcv2b[:, 0:V])
        rCs = work.tile([128, V], bf16, tag="cbfs2", name="rCs")
        nc.vector.tensor_copy(rCs[:], rcv2b[:, V:2 * V])
        retr_fm = work.tile([V, NT], bf16, tag="retrfm", name="retrfm")
        for j in range(NCH):
            ch = slice(128 * j, 128 * (j + 1))
            ps_r2 = pkv.tile([128, V], f32, tag="pr")
            nc.tensor.matmul(ps_r2[:], Qc[:, ch], rCc[:],
                             start=True, stop=False)
            nc.tensor.matmul(ps_r2[:], Qs[:, ch], rCs[:],
                             start=False, stop=True)
            t = work.tile([128, V], bf16, tag="rsc", name=f"rsc{j}")
            nc.vector.tensor_tensor(t[:], ps_r2[:],
                                    retr_sb[:, V * j:V * (j + 1)], Alu.add)
            nc.gpsimd.tensor_scalar(t[:], t[:], rstd_g_tm[:, j:j + 1],
                                    None, Alu.mult)
            ps_f = psm.tile([V, 128], bf16, tag="tr2")
            nc.tensor.transpose(ps_f[:], t[:], eyeb[:])
            nc.scalar.copy(retr_fm[:, ch], ps_f[:])
        dbg("retr_fm", retr_fm[:], 0)

        kvo_w = work.tile([V, D], bf16, tag="wkvo", name="wkvo")
        nc.sync.dma_start(out=kvo_w[:], in_=wts["wT_kvo"][:])
        for o in range(ND):
            for blk in range(NBLK):
                cs = slice(TB * blk, TB * (blk + 1))
                ps = pb.tile([128, TB], f32, tag="lin")
                nc.tensor.matmul(ps[:], kvo_w[:, 128 * o:128 * (o + 1)],
                                 retr_fm[:, cs], start=True, stop=True)
                nc.scalar.activation(pc[2][o][:, cs], ps[:], A.Identity,
                                     bias=bc("kvo_b", o))

        for p in range(5):
            for d in range(ND):
                dbg(f"pc{p}", pc[p][d][:], d)

        pkv.release()
        work.release()

        # ======== P6: LN + o1 + o2 (fresh pool) ========
        p6 = tc.alloc_tile_pool(name="p6", bufs=1)
        pst = tc.alloc_tile_pool(name="pst", bufs=1, space="PSUM")

        # per-d sum of the 5 pieces (tree, alternating engines)
        psum5 = []
        for d in range(ND):
            a = p6.tile([128, NT], bf16, tag=f"p5a{d}", name=f"p5a{d}")
            nc.gpsimd.tensor_tensor(a[:], pc[0][d][:], pc[1][d][:], Alu.add)
            b = p6.tile([128, NT], bf16, tag=f"p5b{d}", name=f"p5b{d}")
            nc.vector.tensor_tensor(b[:], pc[2][d][:], pc[3][d][:], Alu.add)
            nc.gpsimd.tensor_tensor(a[:], a[:], b[:], Alu.add)
            nc.vector.tensor_tensor(a[:], a[:], pc[4][d][:], Alu.add)
            psum5.append(a)

        m_row = p6.tile([1, NT], f32, tag="mrow", name="mrow")
        v_row = p6.tile([1, NT], f32, tag="vrow", name="vrow")
        ps_mean = pst.tile([1, NT], f32, tag="stat")
        for blk in range(NBLK):
            cs = slice(TB * blk, TB * (blk + 1))
            for i, pt in enumerate(psum5):
                nc.tensor.matmul(ps_mean[0:1, cs], onesb[:], pt[:, cs],
                                 start=(i == 0), stop=(i == len(psum5) - 1))
            nc.vector.tensor_scalar(m_row[:, cs], ps_mean[0:1, cs],
                                    1.0 / (5 * D), None, Alu.mult)
        ps_sq = pst.tile([1, NT], f32, tag="stat")
        pieces = [pc[p][d] for p in range(5) for d in range(ND)]
        for blk in range(NBLK):
            cs = slice(TB * blk, TB * (blk + 1))
            for i, pt in enumerate(pieces):
                sq = p6.tile([128, TB], bf16, tag="sqbuf", bufs=2,
                             name=f"sq{blk}_{i}")
                eng = nc.gpsimd if i % 2 == 0 else nc.vector
                eng.tensor_tensor(sq[:], pt[:, cs], pt[:, cs], Alu.mult)
                nc.tensor.matmul(ps_sq[0:1, cs], onesb[:], sq[:],
                                 start=(i == 0), stop=(i == len(pieces) - 1))
            msq = p6.tile([1, TB], f32, tag="msq", name=f"msq{blk}")
            nc.vector.tensor_tensor(msq[:], m_row[0:1, cs], m_row[0:1, cs],
                                    Alu.mult)
            nc.vector.scalar_tensor_tensor(v_row[:, cs], ps_sq[0:1, cs],
                                           1.0 / (5 * D), msq[:],
                                           Alu.mult, Alu.subtract)
        dbg("ln_m", m_row[:], 0, pool=p6)
        dbg("ln_v", v_row[:], 0, pool=p6)
        # [table set B]
        rstd_row = p6.tile([1, NT], bf16, tag="rstdrow", name="rstdrow")
        nc.scalar.activation(v_row[:], v_row[:], A.Ln,
                             bias=bc("eps_ln", rows=1))
        nc.scalar.activation(rstd_row[:], v_row[:], A.Exp, scale=-0.5)
        mrstd_row = p6.tile([1, NT], bf16, tag="mrstdrow", name="mrstdrow")
        nc.vector.tensor_tensor(mrstd_row[:], m_row[:], rstd_row[:], Alu.mult)
        rstd_bc = p6.tile([128, NT], bf16, tag="rstdbc", name="rstdbc")
        mrstd_bc = p6.tile([128, NT], bf16, tag="mrstdbc", name="mrstdbc")
        for blk in range(NBLK):
            cs = slice(TB * blk, TB * (blk + 1))
            psb = pb.tile([128, TB], f32, tag="lin")
            nc.tensor.matmul(psb[:], ones_r1[:], rstd_row[0:1, cs],
                             start=True, stop=True)
            nc.scalar.copy(rstd_bc[:, cs], psb[:])
            psb2 = pb.tile([128, TB], f32, tag="lin")
            nc.tensor.matmul(psb2[:], ones_r1[:], mrstd_row[0:1, cs],
                             start=True, stop=True)
            nc.scalar.copy(mrstd_bc[:, cs], psb2[:])

        # o1 [table set A]
        h1 = [p6.tile([128, NT], bf16, tag=f"h1{o}", name=f"h1{o}")
              for o in range(2 * ND)]
        for o in range(2 * ND):
            o1rows = []
            for i in range(5 * ND):
                t = p6.tile([128, 128], bf16, tag="wo1", bufs=6,
                            name=f"o1r{o}_{i}")
                nc.sync.dma_start(
                    out=t[:],
                    in_=wts["wT_o1"][128 * i:128 * (i + 1),
                                     128 * o:128 * (o + 1)])
                o1rows.append(t)
            for blk in range(NBLK):
                cs = slice(TB * blk, TB * (blk + 1))
                ps = pb.tile([128, TB], f32, tag="lin")
                for i, pt in enumerate(pieces):
                    nc.tensor.matmul(ps[:], o1rows[i][:], pt[:, cs],
                                     start=(i == 0),
                                     stop=(i == len(pieces) - 1))
                h1pre = p6.tile([128, TB], f32, tag="h1pre",
                                name=f"h1p{o}_{blk}")
                nc.vector.tensor_tensor(h1pre[:], ps[:], rstd_bc[:, cs],
                                        Alu.mult)
                mneg = p6.tile([128, TB], f32, tag="mneg",
                               name=f"mneg{o}_{blk}")
                nc.gpsimd.tensor_scalar(mneg[:], mrstd_bc[:, cs],
                                        bc("negw", o), None, Alu.mult)
                nc.gpsimd.tensor_tensor(h1pre[:], h1pre[:], mneg[:], Alu.add)
                nc.scalar.activation(h1[o][:, cs], h1pre[:], A.Gelu,
                                     bias=bc("o1_b", o))
        for d in range(ND):
            dbg("h1", h1[d][:], d, pool=p6)

        o2rows = []
        for i in range(2 * ND):
            t = p6.tile([128, D], bf16, tag="wo2", bufs=8, name=f"o2r{i}")
            nc.sync.dma_start(out=t[:],
                              in_=wts["wT_o2"][128 * i:128 * (i + 1), :])
            o2rows.append(t)
        o2b_sb = p6.tile([1, D], bf16, tag="o2b", name="o2b")
        nc.sync.dma_start(out=o2b_sb[:], in_=wts["o2b_row"][:])
        for j in range(NCH):
            ch = slice(128 * j, 128 * (j + 1))
            ps = pb.tile([128, D], f32, tag="lin")
            for i in range(2 * ND):
                nc.tensor.matmul(ps[:], h1[i][:, ch], o2rows[i][:],
                                 start=(i == 0), stop=False)
            nc.tensor.matmul(ps[:], ones_r1[:], o2b_sb[:],
                             start=False, stop=True)
            xres = p6.tile([128, D], f32, tag="xres", bufs=2,
                           name=f"xres{j}")
            nc.sync.dma_start(out=xres[:],
                              in_=xres_in[128 * j:128 * (j + 1), :])
            out_sb = p6.tile([128, D], f32, tag="outsb", bufs=2,
                             name=f"out{j}")
            nc.vector.tensor_tensor(out_sb[:], ps[:], xres[:], Alu.add)
            nc.sync.dma_start(out=y_out[128 * j:128 * (j + 1), :],
                              in_=out_sb[:])

        pst.release()
        p6.release()
        dram.release()
        psm.release()
        pb.release()
        keep.release()
        con.release()

    fixup_excess_waits(nc)
    return nc, dbg_shapes


# ===================== host side =====================

def _prep_host(inputs):
    import ml_dtypes
    bft = ml_dtypes.bfloat16
    g = {k: np.asarray(v, dtype=np.float32) for k, v in inputs.items()}
    c = float(np.abs(g["mag_scale"]))
    absw = np.abs(g["omega_scale"])

    def pack4(wT, width):
        return np.ascontiguousarray(
            wT.reshape(ND, 128, width).transpose(1, 0, 2).reshape(
                128, ND * width))

    W = {}
    W["wT_tw"] = (g["tw_w"] * absw[:, None]).T
    W["wT_pi0"] = g["pi0_w"].T
    W["wT_pi2"] = g["pi2_w"].T
    W["wT_m1v"] = (g["m1v_w"] * c).T
    W["wT_mag"] = g["mag_w"].T
    W["wT_qo"] = g["qo_w"].T
    W["wT_cp"] = g["cp_w"].T
    W["wT_m1o"] = (g["m1o_w"] / math.sqrt(D)).T
    W["kepack"] = pack4(g["ke_w"].T, 128)
    W["vepack"] = pack4(g["ve_w"].T, V)
    W["sgpack"] = pack4(g["sg_w"].T, 1)
    W["wT_sk0"] = g["sk0_w"].T
    W["sk2pack"] = pack4(g["sk2_w"].T, 128)
    W["wT_kvo"] = (g["kvo_w"] / math.sqrt(P)).T
    o1w = g["o1_w"] * g["ln_g"][None, :]
    W["wT_o1"] = o1w.T
    W["wT_o2"] = g["o2_w"].T
    W["o2b_row"] = g["o2_b"][None, :]
    W["ones_col"] = np.ones((128, 1), np.float32)
    W["ones_row1"] = np.ones((1, 128), np.float32)
    W["eye_b"] = np.eye(128, dtype=np.float32)
    W["tril_b"] = np.triu(np.ones((128, 128), np.float32))
    W = {k: np.ascontiguousarray(v).astype(bft) for k, v in W.items()}

    negWsum = -o1w.sum(axis=1)
    b1p = g["o1_b"] + g["o1_w"] @ g["ln_b"]

    bias = np.zeros((128, NBIAS), np.float32)
    def put(name, vec, i=0):
        v = np.asarray(vec, np.float32).ravel()
        bias[:len(v), BC[name] + i] = v
    for d in range(ND):
        sl = slice(128 * d, 128 * (d + 1))
        put("tw_b", (g["tw_b"] * absw)[sl], d)
        put("pi0_b", g["pi0_b"][sl], d)
        put("pi2_b", g["pi2_b"][sl], d)
        put("m1v_b", (g["m1v_b"] * c)[sl], d)
        put("mag_b", (0.5 * g["mag_b"])[sl], d)
        put("qo_b", g["qo_b"][sl], d)
        put("cp_b", g["cp_b"][sl], d)
        put("m1o_b", g["m1o_b"][sl], d)
        put("sk0_b", g["sk0_b"][sl], d)
        put("kvo_b", g["kvo_b"][sl], d)
        put("lc_b", g["lc_b"][sl], d)
        put("cg_b", (0.5 * g["cg_b"])[sl], d)
        for k in range(K):
            put("lc_w", g["lc_w"][sl, 0, k], 4 * d + k)
            put("cg_w", g["cg_w"][sl, 0, k], 4 * d + k)
    put("ke_b", g["ke_b"])
    put("ve_b", g["ve_b"])
    put("sg_b", 0.5 * g["sg_b"])
    put("sk2_b", g["sk2_b"])
    for o in range(8):
        put("o1_b", b1p[128 * o:128 * (o + 1)], o)
        put("negw", negWsum[128 * o:128 * (o + 1)], o)
    put("halfpi", np.full(128, HALF_PI))
    put("eps_mag", np.full(128, 1e-8))
    put("c_mag", np.full(128, c))
    put("eps_ln", np.full(128, 1e-5))

    pos = np.arange(1, L + 1, dtype=np.float32)

    x = g["x"]
    in_maps = []
    for core in range(N_CORES):
        b, h = core // 2, core % 2
        xe = np.zeros((NT + 3, D), np.float32)
        if h == 0:
            xe[3:] = x[b, 0:NT]
        else:
            xe[:] = x[b, NT - 3:2 * NT]
        xT = np.ascontiguousarray(xe.T).astype(bft)
        rp = np.broadcast_to(1.0 / pos[h * NT:(h + 1) * NT][None, :],
                             (128, NT)).astype(bft)
        m = {"xT": xT, "x_res": np.ascontiguousarray(xe[3:]),
             "bias_pack": bias, "recip_pos": rp,
             "eye_f": np.eye(V, dtype=np.float32),
             "send_mask": np.full((128, 1), 1.0 - h, np.float32),
             "use_mask": np.full((128, 1), float(h), np.float32)}
        m.update(W)
        in_maps.append(m)
    return in_maps


_CACHE = {}

def _get_built(debug=()):
    key = tuple(sorted(debug))
    if key not in _CACHE:
        _CACHE[key] = build_nc(key)
    return _CACHE[key]


def run_cores(inputs, debug=(), trace=False):
    from concourse.bass_utils import run_bass_kernel_spmd
    nc, dbg_shapes = _get_built(debug)
    in_maps = _prep_host(inputs)
    res = run_bass_kernel_spmd(nc, in_maps, list(range(N_CORES)),
                               trace=trace)
    return res


def kernel(**inputs):
    results = run_cores(inputs).results
    out = np.empty((B, L, D), np.float32)
    for core in range(N_CORES):
        b, h = core // 2, core % 2
        out[b, h * NT:(h + 1) * NT] = results[core]["y"]
    return out


# revision 34
# speedup vs baseline: 1.7880x; 1.7187x over previous
"""Trainium2 Bass kernel for nn_EvolvingLocalConvBlock (v3, bf16-resident).

Sharding: 8 cores = 4 samples x 2 sequence halves (1024 tokens each).
Cross-core cumsum carries via three pairwise AllReduces (even core sends
masked totals; odd core consumes).

v3 design:
- No DRAM staging: all [128, NT] intermediates live in SBUF in bf16
  (scans keep f32 internal state regardless of operand dtype).
- x uploaded pre-transposed (feature-major) bf16: no PE transposes.
- All weights bf16; all big matmuls bf16 at full PE rate.
- Depthwise conv done on PE via diagonal-matmul taps (4 accumulating
  matmuls per branch/feature-group/block).
- GpSimd(Pool) engine used ONLY for collectives (its elementwise rate
  is ~10x slower than DVE).
- Direct Sigmoid/Rsqrt activations; table-set order
  sigmoid -> gelu -> rsqrt -> trig -> rsqrt -> gelu (6 loads).
- LN mean folded into o1 via a negw-row matmul; LN stats row ops done
  in chunk-transposed [128, NCH] space (single-partition DVE ops are
  pathologically slow).
- Collectives hidden behind independent work (conv/pi2/qo | kv loop |
  mem1+m1o).
"""
import sys
sys.path.insert(0, '/opt/trn_rl_repo')

import math
import numpy as np

import concourse.bass as bass
import concourse.mybir as mybir
from concourse.tile import TileContext

B, L, D, P, V, K = 4, 2048, 512, 128, 8, 4
N_CORES = 8
NT = L // 2
NCH = NT // 128
ND = D // 128
NBLK = 2
TB = NT // NBLK

f32 = mybir.dt.float32
bf16 = mybir.dt.bfloat16
A = mybir.ActivationFunctionType
Alu = mybir.AluOpType

TWO_PI = 2.0 * math.pi
HALF_PI = math.pi / 2.0

# ---- bias_pack column map ----
BC = {}
_ncols = 0
def _bc(name, n):
    global _ncols
    BC[name] = _ncols
    _ncols += n
for _n, _k in [("tw_b", ND), ("pi0_b", ND), ("pi2_b", ND), ("m1v_b", ND),
               ("mag_b", ND), ("qo_b", ND), ("cp_b", ND), ("m1o_b", ND),
               ("ke_b", 1), ("ve_b", 1), ("sg_b", 1), ("sk0_b", ND),
               ("sk2_b", 1), ("kvo_b", ND), ("o1_b", 8),
               ("lc_b", ND), ("cg_b", ND),
               ("halfpi", 1), ("eps_mag", 1), ("c_mag", 1), ("eps_ln", 1)]:
    _bc(_n, _k)
NBIAS = _ncols


def fixup_excess_waits(nc, max_waits=1, max_updates=1):
    """This walrus accepts at most one sync wait/update per instruction;
    hoist extras onto adjacent same-engine NoOps."""
    for f in nc.m.functions:
        for bb in f.blocks:
            new = []
            changed = False
            for ins in bb.instructions:
                si = getattr(ins, 'sync_info', None)
                if si is None:
                    new.append(ins)
                    continue
                w = list(si.on_wait) if si.on_wait else []
                if len(w) > max_waits:
                    excess, keep = w[:-max_waits], w[-max_waits:]
                    for i in range(0, len(excess), max_waits):
                        nop = mybir.InstNoOp(name=f"{ins.name}-hw{i}",
                                             engine=ins.engine, ins=[], outs=[])
                        nop.sync_info = mybir.SyncInfo(
                            on_wait=excess[i:i + max_waits], on_update=[])
                        new.append(nop)
                    si.on_wait = keep
                    changed = True
                new.append(ins)
                u = list(si.on_update) if si.on_update else []
                if len(u) > max_updates:
                    excess_u, keep_u = u[max_updates:], u[:max_updates]
                    for i in range(0, len(excess_u), max_updates):
                        nop = mybir.InstNoOp(name=f"{ins.name}-hu{i}",
                                             engine=ins.engine, ins=[], outs=[])
                        nop.sync_info = mybir.SyncInfo(
                            on_wait=[], on_update=excess_u[i:i + max_updates])
                        new.append(nop)
                    si.on_update = keep_u
                    changed = True
            if changed:
                bb.instructions = new


def build_nc(debug=()):
    nc = bass.Bass()
    dp = nc.declare_dram_parameter

    xT_in = dp("xT", [D, NT + 3], bf16, isOutput=False)
    xres_in = dp("x_res", [NT, D], f32, isOutput=False)
    y_out = dp("y", [NT, D], f32, isOutput=True)

    wts = {}
    for name, shape in [
        ("wT_tw", [D, D]), ("wT_pi0", [D, D]), ("wT_pi2", [D, D]),
        ("wT_m1v", [D, D]), ("wT_mag", [D, D]), ("wT_qo", [D, D]),
        ("wT_cp", [D, D]), ("wT_m1o", [D, D]),
        ("kepack", [128, ND * 128]), ("vepack", [128, ND * V]),
        ("sgpack", [128, ND]), ("wT_sk0", [2 * D, D]),
        ("sk2pack", [128, ND * 128]), ("wT_kvo", [V, D]),
        ("wT_o1", [5 * D, 2 * D]), ("wT_o2", [2 * D, D]),
        ("o2b_row", [1, D]), ("negw_row", [1, 2 * D]),
        ("ones_col", [128, 1]), ("ones_row1", [1, 128]),
        ("eye_b", [128, 128]), ("tril_b", [128, 128]),
        ("recip_pos", [128, NT]),
        ("convdiag", [128, 2 * ND * K * 128]),
    ]:
        wts[name] = dp(name, shape, bf16, isOutput=False)
    eyef_in = dp("eye_f", [V, V], f32, isOutput=False)
    eyeff_in = dp("eye_ff", [128, 128], f32, isOutput=False)
    bias_in = dp("bias_pack", [128, NBIAS], f32, isOutput=False)
    smask_in = dp("send_mask", [128, 1], f32, isOutput=False)
    umask_in = dp("use_mask", [128, 1], f32, isOutput=False)

    dbg_shapes = {}
    RG = [[0, 1], [2, 3], [4, 5], [6, 7]]

    with TileContext(nc) as tc:
        con = tc.alloc_tile_pool(name="con", bufs=1, side="left")
        keep = tc.alloc_tile_pool(name="keep", bufs=1, side="left")
        work = tc.alloc_tile_pool(name="work", bufs=1)
        pb = tc.alloc_tile_pool(name="pb", bufs=4, space="PSUM")
        psm = tc.alloc_tile_pool(name="psm", bufs=1, space="PSUM")
        pkv = tc.alloc_tile_pool(name="pkv", bufs=1, space="PSUM")
        dram = tc.alloc_tile_pool(name="dram", bufs=1, space="DRAM")

        dbg_bufs = {}
        def dbg(name, ap, part, pool=None):
            """Dump (rows, NT) AP into 128-row slot `part` of a debug out."""
            if name not in debug:
                return
            pool = pool or work
            r = ap.shape[0]
            if name not in dbg_bufs:
                dbg_bufs[name] = dp("dbg_" + name, [ND * 128, NT], f32,
                                    isOutput=True)
                dbg_shapes[name] = True
            t = dbg_bufs[name]
            if ap.dtype != f32:
                cv = pool.tile([r, ap.shape[1]], f32, tag="dbgcv",
                               name=f"dbgcv{name}{part}")
                nc.vector.tensor_copy(cv[:], ap)
                ap = cv[:]
            nc.sync.dma_start(out=t[128 * part:128 * part + r, :], in_=ap)

        # ---------------- constants ----------------
        bias = con.tile([128, NBIAS], f32, tag="bias")
        nc.sync.dma_start(out=bias[:], in_=bias_in[:])
        def bc(name, i=0, rows=128):
            return bias[0:rows, BC[name] + i:BC[name] + i + 1]
        eyeb = con.tile([128, 128], bf16, tag="eyeb")
        nc.sync.dma_start(out=eyeb[:], in_=wts["eye_b"][:])
        trilb = con.tile([128, 128], bf16, tag="trilb")
        nc.sync.dma_start(out=trilb[:], in_=wts["tril_b"][:])
        eyef = con.tile([V, V], f32, tag="eyef")
        nc.sync.dma_start(out=eyef[:], in_=eyef_in[:])
        eyeff = con.tile([128, 128], f32, tag="eyeff")
        nc.sync.dma_start(out=eyeff[:], in_=eyeff_in[:])
        smask = con.tile([128, 1], f32, tag="smask")
        nc.sync.dma_start(out=smask[:], in_=smask_in[:])
        umask = con.tile([128, 1], f32, tag="umask")
        nc.sync.dma_start(out=umask[:], in_=umask_in[:])
        onesb = con.tile([128, 1], bf16, tag="onesb")
        nc.sync.dma_start(out=onesb[:], in_=wts["ones_col"][:])
        ones_r1 = con.tile([1, 128], bf16, tag="onesr1")
        nc.sync.dma_start(out=ones_r1[:], in_=wts["ones_row1"][:])
        negw_sb = con.tile([1, 2 * D], bf16, tag="negwsb")
        nc.sync.dma_start(out=negw_sb[:], in_=wts["negw_row"][:])
        recip = con.tile([128, NT], bf16, tag="recip")
        nc.sync.dma_start(out=recip[:], in_=wts["recip_pos"][:])
        zerosb = con.tile([128, NT], bf16, tag="zerosb")
        nc.vector.memset(zerosb[:], 0.0)
        lastc = con.tile([128, 16], f32, tag="lastc")

        # pieces (live to the end)
        pc = [[keep.tile([128, NT], bf16, tag=f"pc{p}{d}", name=f"pc{p}{d}")
               for d in range(ND)] for p in range(5)]

        # ---------------- P0: x load (already feature-major) ----------
        x_fm = []
        for d in range(ND):
            t = work.tile([128, NT + 3], bf16, tag=f"x{d}", name=f"x{d}")
            nc.sync.dma_start(out=t[:],
                              in_=xT_in[128 * d:128 * (d + 1), :])
            x_fm.append(t)
        xin = [t[:, 3:3 + NT] for t in x_fm]

        # long-lived bf16 stores
        som = [work.tile([128, NT], bf16, tag=f"som{d}", name=f"som{d}")
               for d in range(ND)]
        Ssig = [work.tile([128, NT], bf16, tag=f"ssg{d}", name=f"ssg{d}")
                for d in range(ND)]
        Sx = [work.tile([128, NT], bf16, tag=f"sx{d}", name=f"sx{d}")
              for d in range(ND)]
        def scr(tag):        # f32 [128, NT] scratch
            return work.tile([128, NT], f32, tag=tag, name=tag)
        def scrb(tag):       # bf16 [128, NT] scratch
            return work.tile([128, NT], bf16, tag=tag, name=tag)

        # ---------------- helpers ----------------
        def load_wrows(name, nin, nout, tag="w4", bufs=4):
            rows = []
            for i in range(nin):
                t = work.tile([128, nout], bf16, tag=tag, bufs=bufs,
                              name=f"{name}r{i}")
                nc.sync.dma_start(out=t[:],
                                  in_=wts[name][128 * i:128 * (i + 1), :])
                rows.append(t)
            return rows

        def mm_big(wname, rhs_tiles, epilogue, nout=D, tag="w4", bufs=4):
            rows = load_wrows(wname, len(rhs_tiles), nout, tag=tag, bufs=bufs)
            for o in range(nout // 128):
                for blk in range(NBLK):
                    cs = slice(TB * blk, TB * (blk + 1))
                    ps = pb.tile([128, TB], f32, tag="lin")
                    for i, r in enumerate(rhs_tiles):
                        nc.tensor.matmul(ps[:],
                                         rows[i][:, 128 * o:128 * (o + 1)],
                                         r[:, cs], start=(i == 0),
                                         stop=(i == len(rhs_tiles) - 1))
                    epilogue(o, blk, ps)

        def mm_packed(wname, rhs_tiles, out_rows, epilogue):
            nin = len(rhs_tiles)
            wrow = work.tile([128, nin * out_rows], bf16, tag="wp1",
                             bufs=2, name=wname)
            nc.sync.dma_start(out=wrow[:], in_=wts[wname][:])
            for blk in range(NBLK):
                cs = slice(TB * blk, TB * (blk + 1))
                ps = pb.tile([out_rows, TB], f32, tag="lin")
                for i in range(nin):
                    nc.tensor.matmul(ps[:],
                                     wrow[:, out_rows * i:out_rows * (i + 1)],
                                     rhs_tiles[i][:, cs],
                                     start=(i == 0), stop=(i == nin - 1))
                epilogue(blk, ps)

        def scan_dve(dst_ap, src_ap, rows=128):
            nc.vector.tensor_tensor_scan(dst_ap, zerosb[0:rows, 0:NT],
                                         src_ap, 0.0, Alu.add, Alu.add)

        def exchange(n, fill):
            pk = work.tile([128, n], f32, tag="pk", name=f"pk{n}")
            nc.vector.memset(pk[:], 0.0)
            fill(pk)
            cin = dram.tile([128, n], f32, tag=f"ci{n}")
            cout = dram.tile([128, n], f32, tag=f"co{n}")
            nc.sync.dma_start(out=cin[:], in_=pk[:])
            nc.gpsimd.collective_compute(
                "AllReduce", Alu.add, replica_groups=RG,
                ins=[cin.opt()], outs=[cout.opt()])
            rcv = work.tile([128, n], f32, tag=f"rc{n}", name=f"rc{n}")
            nc.sync.dma_start(out=rcv[:], in_=cout[:])
            rcvu = work.tile([128, n], f32, tag=f"ru{n}", name=f"ru{n}")
            nc.vector.tensor_scalar(rcvu[:], rcv[:], umask[:, 0:1], None,
                                    Alu.mult)
            return rcvu

        # ======== P1a [sigmoid table set]: tw/mag/sg/ke -> exchange1 ====
        for d in range(ND):
            scan_dve(Sx[d][:], xin[d])
            nc.vector.tensor_copy(lastc[:, 8 + d:9 + d], Sx[d][:, NT - 1:NT])

        twrows = load_wrows("wT_tw", ND, D)
        for o in range(ND):
            omt = scrb("bA")
            for blk in range(NBLK):
                cs = slice(TB * blk, TB * (blk + 1))
                ps = pb.tile([128, TB], f32, tag="lin")
                for i in range(ND):
                    nc.tensor.matmul(ps[:], twrows[i][:, 128 * o:128 * (o + 1)],
                                     xin[i][:, cs], start=(i == 0),
                                     stop=(i == ND - 1))
                nc.scalar.activation(omt[:, cs], ps[:], A.Identity,
                                     bias=bc("tw_b", o))
            scan_dve(som[o][:], omt[:])
            nc.vector.tensor_copy(lastc[:, o:o + 1], som[o][:, NT - 1:NT])

        sig = [work.tile([128, NT], bf16, tag=f"sig{d}", name=f"sig{d}")
               for d in range(ND)]
        def ep_sig(o, blk, ps):
            nc.scalar.activation(sig[o][:, TB * blk:TB * (blk + 1)], ps[:],
                                 A.Sigmoid, bias=bc("mag_b", o))
        mm_big("wT_mag", xin, ep_sig)
        for d in range(ND):
            scan_dve(Ssig[d][:], sig[d][:])
            nc.vector.tensor_copy(lastc[:, 4 + d:5 + d],
                                  Ssig[d][:, NT - 1:NT])
            dbg("sig", sig[d][:], d)

        gate = work.tile([1, NT], bf16, tag="gate", name="gate")
        def ep_sg(blk, ps):
            nc.scalar.activation(gate[:, TB * blk:TB * (blk + 1)], ps[:],
                                 A.Sigmoid, bias=bc("sg_b", rows=1))
        mm_packed("sgpack", xin, 1, ep_sg)
        S_gate = work.tile([1, NT], f32, tag="sgate", name="sgate")
        scan_dve(S_gate[:], gate[:], rows=1)

        t_ke = work.tile([128, NT], bf16, tag="tke", name="tke")
        def ep_ke(blk, ps):
            nc.scalar.activation(t_ke[:, TB * blk:TB * (blk + 1)], ps[:],
                                 A.Tanh, bias=bc("ke_b"))
        mm_packed("kepack", xin, 128, ep_ke)

        def fill1(pk):
            for c in range(12):
                nc.vector.tensor_scalar(pk[:, c:c + 1], lastc[:, c:c + 1],
                                        smask[:, 0:1], None, Alu.mult)
            nc.vector.tensor_scalar(pk[0:1, 12:13], S_gate[:, NT - 1:NT],
                                    smask[0:1, 0:1], None, Alu.mult)
        rcv1 = exchange(13, fill1)

        # ======== P1b (exchange1 in flight) ========
        # depthwise conv on PE via diagonal taps (branch-sequential so the
        # diag-weight tile is reused between branches)
        co_t = [scrb(f"cvc{d}") for d in range(ND)]
        for br in range(2):
            convw = work.tile([128, ND * K * 128], bf16, tag="convw",
                              name=f"convw{br}")
            nc.sync.dma_start(
                out=convw[:],
                in_=wts["convdiag"][:, br * ND * K * 128:
                                    (br + 1) * ND * K * 128])
            for d in range(ND):
                dst = co_t[d] if br == 0 else scrb("bB")
                for blk in range(NBLK):
                    ps = pb.tile([128, TB], f32, tag="lin")
                    for k in range(K):
                        off = (d * K + k) * 128
                        nc.tensor.matmul(
                            ps[:], convw[:, off:off + 128],
                            x_fm[d][:, k + blk * TB:k + blk * TB + TB],
                            start=(k == 0), stop=(k == K - 1))
                    if br == 0:
                        nc.scalar.activation(
                            dst[:, TB * blk:TB * (blk + 1)], ps[:],
                            A.Identity, bias=bc("lc_b", d))
                    else:
                        nc.scalar.activation(
                            dst[:, TB * blk:TB * (blk + 1)], ps[:],
                            A.Sigmoid, bias=bc("cg_b", d))
                if br == 1:
                    gt = co_t[d]    # overwrite co with gated product
                    nc.vector.tensor_tensor(gt[:], dst[:], co_t[d][:],
                                            Alu.mult)
                    dbg("convg", gt[:], d)
        convg = co_t

        def ep_store(dst_list, bname):
            def ep(o, blk, ps):
                nc.scalar.activation(
                    dst_list[o][:, TB * blk:TB * (blk + 1)], ps[:],
                    A.Identity, bias=bc(bname, o))
            return ep
        mm_big("wT_cp", [t[:] for t in convg], ep_store(pc[0], "cp_b"))

        v1 = [work.tile([128, NT], bf16, tag=f"v1{d}", name=f"v1{d}")
              for d in range(ND)]
        mm_big("wT_m1v", xin, ep_store(v1, "m1v_b"))

        vals = work.tile([V, NT], bf16, tag="vals", name="vals")
        def ep_ve(blk, ps):
            nc.scalar.activation(vals[:, TB * blk:TB * (blk + 1)], ps[:],
                                 A.Identity, bias=bc("ve_b", rows=V))
        mm_packed("vepack", xin, V, ep_ve)

        # [switch to gelu table set]
        g0 = [work.tile([128, NT], bf16, tag=f"g0{d}", name=f"g0{d}")
              for d in range(ND)]
        def ep_g0(o, blk, ps):
            nc.scalar.activation(g0[o][:, TB * blk:TB * (blk + 1)], ps[:],
                                 A.Gelu, bias=bc("pi0_b", o))
        mm_big("wT_pi0", xin, ep_g0)

        pi2o = [work.tile([128, NT], bf16, tag=f"pio{d}", name=f"pio{d}")
                for d in range(ND)]
        mm_big("wT_pi2", [t[:] for t in g0], ep_store(pi2o, "pi2_b"))

        qoo = [work.tile([128, NT], bf16, tag=f"qoo{d}", name=f"qoo{d}")
               for d in range(ND)]
        mm_big("wT_qo", xin, ep_store(qoo, "qo_b"))

        # ======== P3.1 (needs rcv1) [gelu set: Gelu + Tanh] ========
        sk0rows = load_wrows("wT_sk0", 2 * ND, D, tag="w8", bufs=8)
        gsk = [work.tile([128, NT], bf16, tag=f"gsk{d}", name=f"gsk{d}")
               for d in range(ND)]
        for blk in range(NBLK):
            cs = slice(TB * blk, TB * (blk + 1))
            ctxc = []
            for d in range(ND):
                t = work.tile([128, TB], bf16, tag=f"ctx{d}",
                              name=f"ctx{d}_{blk}")
                nc.vector.scalar_tensor_tensor(t[:], Sx[d][:, cs],
                                               rcv1[:, 8 + d:9 + d],
                                               recip[:, cs],
                                               Alu.add, Alu.mult)
                ctxc.append(t)
            for o in range(ND):
                ps = pb.tile([128, TB], f32, tag="lin")
                for i in range(ND):
                    nc.tensor.matmul(ps[:],
                                     sk0rows[i][:, 128 * o:128 * (o + 1)],
                                     xin[i][:, cs], start=(i == 0),
                                     stop=False)
                for i in range(ND):
                    nc.tensor.matmul(
                        ps[:], sk0rows[ND + i][:, 128 * o:128 * (o + 1)],
                        ctxc[i][:], start=False, stop=(i == ND - 1))
                nc.scalar.activation(gsk[o][:, cs], ps[:], A.Gelu,
                                     bias=bc("sk0_b", o))
        t_sk = work.tile([128, NT], bf16, tag="tsk", name="tsk")
        def ep_sk2(blk, ps):
            nc.scalar.activation(t_sk[:, TB * blk:TB * (blk + 1)], ps[:],
                                 A.Tanh, bias=bc("sk2_b"))
        mm_packed("sk2pack", [t[:] for t in gsk], 128, ep_sk2)

        # ======== P3.2 [rsqrt table set] ========
        rstdm = []
        for d in range(ND):
            bcol = work.tile([128, 1], f32, tag="rsb", name=f"rsb{d}")
            nc.vector.tensor_scalar(bcol[:], rcv1[:, 4 + d:5 + d],
                                    bc("c_mag"), bc("eps_mag"),
                                    Alu.mult, Alu.add)
            rf = scr("sA")
            nc.scalar.activation(rf[:], Ssig[d][:], A.Ln,
                                 bias=bcol[:, 0:1], scale=bc("c_mag"))
            t = work.tile([128, NT], bf16, tag=f"rsm{d}", name=f"rsm{d}")
            nc.scalar.activation(t[:], rf[:], A.Exp, scale=-0.5)
            rstdm.append(t)
            dbg("rstdm", t[:], d)

        # gate norm: max(S+c, 1) via Relu(S + (c-1)) + 1, then rsqrt
        gb = work.tile([1, 1], f32, tag="gb", name="gb")
        nc.vector.tensor_scalar(gb[:], rcv1[0:1, 12:13], 1.0, -1.0,
                                Alu.mult, Alu.add)
        gn_row = S_gate
        nc.scalar.activation(gn_row[:], S_gate[:], A.Relu, bias=gb[0:1, 0:1])
        nc.scalar.activation(gn_row[:], gn_row[:], A.Ln, bias=1.0)
        nc.scalar.activation(gn_row[:], gn_row[:], A.Exp, scale=-0.5)
        dbg("gnr", gn_row[:], 0)
        rstd_g_tm = work.tile([128, NCH], f32, tag="rgtm", name="rgtm")
        for j in range(NCH):
            psj = psm.tile([128, 1], f32, tag="tr")
            nc.tensor.transpose(psj[:], gn_row[0:1, 128 * j:128 * (j + 1)],
                                eyef[0:1, 0:1])
            nc.scalar.copy(rstd_g_tm[:, j:j + 1], psj[:])

        # ======== P3.3 [trig table set] ========
        def phase_cs(tin, cname, sname):
            s_t = work.tile([128, NT], bf16, tag=sname, name=sname)
            nc.scalar.activation(s_t[:], tin[:], A.Sin, scale=math.pi)
            m = scr("sB")
            nc.vector.tensor_scalar(m[:], tin[:], 0.5, None, Alu.is_gt)
            nc.vector.scalar_tensor_tensor(m[:], m[:], -2.0, tin[:],
                                           Alu.mult, Alu.add)
            c_t = work.tile([128, NT], bf16, tag=cname, name=cname)
            nc.scalar.activation(c_t[:], m[:], A.Sin, scale=math.pi,
                                 bias=bc("halfpi"))
            return c_t, s_t
        Qc, Qs = phase_cs(t_ke, "Qc", "Qs")
        Kc, Ks = phase_cs(t_sk, "Kc", "Ks")
        dbg("Qc", Qc[:], 0)
        dbg("Kc", Kc[:], 0)

        cosq, sinq, posret = [], [], []
        for o in range(ND):
            phic = scr("sA")
            nc.vector.scalar_tensor_tensor(phic[:], som[o][:],
                                           rcv1[:, o:o + 1], pi2o[o][:],
                                           Alu.add, Alu.add)
            dbg("phi", phic[:], o)
            cphi = scrb("bB")
            nc.scalar.activation(cphi[:], phic[:], A.Sin, bias=bc("halfpi"))
            sphi = scrb("bC")
            nc.scalar.activation(sphi[:], phic[:], A.Sin)
            nc.vector.tensor_tensor(pc[3][o][:], xin[o], cphi[:], Alu.mult)
            nc.vector.tensor_tensor(pc[4][o][:], xin[o], sphi[:], Alu.mult)
            wv = scrb("bD")
            nc.vector.tensor_tensor(wv[:], sig[o][:], v1[o][:], Alu.mult)
            nc.vector.tensor_tensor(cphi[:], wv[:], cphi[:], Alu.mult)
            scan_dve(Ssig[o][:], cphi[:])      # Ssig becomes Sc
            nc.vector.tensor_copy(lastc[:, 4 + o:5 + o],
                                  Ssig[o][:, NT - 1:NT])
            dbg("Sc", Ssig[o][:], o)
            nc.vector.tensor_tensor(sphi[:], wv[:], sphi[:], Alu.mult)
            scan_dve(Sx[o][:], sphi[:])        # Sx becomes Ss
            nc.vector.tensor_copy(lastc[:, 8 + o:9 + o],
                                  Sx[o][:, NT - 1:NT])
            # phi_q = phi + qo_out (qo bias already applied)
            nc.vector.tensor_tensor(phic[:], phic[:], qoo[o][:], Alu.add)
            sq_t = v1[o]                        # reuse v1 tile for sinq
            nc.scalar.activation(sq_t[:], phic[:], A.Sin)
            sinq.append(sq_t)
            m = scrb("bD")
            nc.vector.tensor_scalar(m[:], phic[:], HALF_PI, None, Alu.is_gt)
            nc.vector.scalar_tensor_tensor(phic[:], m[:], -TWO_PI, phic[:],
                                           Alu.mult, Alu.add)
            cq_t = g0[o]                        # reuse g0 tile for cosq
            nc.scalar.activation(cq_t[:], phic[:], A.Sin, bias=bc("halfpi"))
            cosq.append(cq_t)
            dbg("cosq", cq_t[:], o)

        def fill2a(pk):
            for c in range(8):
                nc.vector.tensor_scalar(pk[:, c:c + 1], lastc[:, 4 + c:5 + c],
                                        smask[:, 0:1], None, Alu.mult)
        rcv2a = exchange(8, fill2a)

        # ======== P3.4 (exchange2a in flight): kv chunk loop ========
        retr_sb = work.tile([128, V * NCH], f32, tag="retr", name="retr")
        Cbf_c = work.tile([128, V], bf16, tag="cbfc", name="cbfc")
        Cbf_s = work.tile([128, V], bf16, tag="cbfs", name="cbfs")
        for j in range(NCH):
            ch = slice(128 * j, 128 * (j + 1))
            ps_st = psm.tile([128, 128], f32, tag="tr")
            nc.tensor.matmul(ps_st[:], Kc[:, ch], Qc[:, ch],
                             start=True, stop=False)
            nc.tensor.matmul(ps_st[:], Ks[:, ch], Qs[:, ch],
                             start=False, stop=True)
            st_sb = work.tile([128, 128], bf16, tag="stsb", name=f"st{j}")
            nc.vector.tensor_tensor(st_sb[:], ps_st[:], trilb[:], Alu.mult)
            ps_v = psm.tile([128, V + 1], bf16, tag="tr2")
            nc.tensor.transpose(ps_v[:, 0:V], vals[:, ch], eyeb[0:V, 0:V])
            nc.tensor.transpose(ps_v[:, V:V + 1], gate[0:1, ch],
                                eyeb[0:1, 0:1])
            gvcol = work.tile([128, 1], f32, tag="gvcol", name=f"gvc{j}")
            nc.scalar.copy(gvcol[:], ps_v[:, V:V + 1])
            gv = work.tile([128, V], bf16, tag="gv", name=f"gv{j}")
            nc.vector.tensor_scalar(gv[:], ps_v[:, 0:V], gvcol[:, 0:1],
                                    None, Alu.mult)
            ps_r = pkv.tile([128, V], f32, tag="pr")
            nc.tensor.matmul(ps_r[:], st_sb[:], gv[:], start=True,
                             stop=(j == 0))
            if j > 0:
                nc.tensor.matmul(ps_r[:], Qc[:, ch], Cbf_c[:],
                                 start=False, stop=False)
                nc.tensor.matmul(ps_r[:], Qs[:, ch], Cbf_s[:],
                                 start=False, stop=True)
            nc.scalar.copy(retr_sb[:, V * j:V * (j + 1)], ps_r[:])
            ps_kt = psm.tile([128, 128], bf16, tag="tr2")
            nc.tensor.transpose(ps_kt[:], Kc[:, ch], eyeb[:])
            kctm = work.tile([128, 128], bf16, tag="kctm", name=f"kc{j}")
            nc.scalar.copy(kctm[:], ps_kt[:])
            ps_kt2 = psm.tile([128, 128], bf16, tag="tr2")
            nc.tensor.transpose(ps_kt2[:], Ks[:, ch], eyeb[:])
            kstm = work.tile([128, 128], bf16, tag="kstm", name=f"ks{j}")
            nc.scalar.copy(kstm[:], ps_kt2[:])
            ps_cc = pkv.tile([128, 2 * V], f32, tag="cc")
            nc.tensor.matmul(ps_cc[:, 0:V], kctm[:], gv[:],
                             start=True, stop=True)
            nc.tensor.matmul(ps_cc[:, V:2 * V], kstm[:], gv[:],
                             start=True, stop=True)
            if j == 0:
                nc.vector.tensor_copy(Cbf_c[:], ps_cc[:, 0:V])
                nc.vector.tensor_copy(Cbf_s[:], ps_cc[:, V:2 * V])
            else:
                nc.vector.tensor_tensor(Cbf_c[:], Cbf_c[:],
                                        ps_cc[:, 0:V], Alu.add)
                nc.vector.tensor_tensor(Cbf_s[:], Cbf_s[:],
                                        ps_cc[:, V:2 * V], Alu.add)

        def fill2b(pk):
            nc.vector.tensor_scalar(pk[:, 0:V], Cbf_c[:], smask[:, 0:1],
                                    None, Alu.mult)
            nc.vector.tensor_scalar(pk[:, V:2 * V], Cbf_s[:],
                                    smask[:, 0:1], None, Alu.mult)
        rcv2b = exchange(2 * V, fill2b)

        # ======== P3.5 (exchange2b in flight): mem1 + m1o ========
        for d in range(ND):
            t1 = scrb("bB")
            nc.vector.scalar_tensor_tensor(t1[:], Ssig[d][:],
                                           rcv2a[:, d:d + 1],
                                           cosq[d][:], Alu.add, Alu.mult)
            t2 = scrb("bC")
            nc.vector.scalar_tensor_tensor(t2[:], Sx[d][:],
                                           rcv2a[:, 4 + d:5 + d],
                                           sinq[d][:], Alu.add, Alu.mult)
            nc.vector.tensor_tensor(t1[:], t1[:], t2[:], Alu.add)
            pr = sig[d]                         # reuse sig tile for pos_ret
            nc.vector.tensor_tensor(pr[:], t1[:], rstdm[d][:], Alu.mult)
            posret.append(pr)
            dbg("pos_ret", pr[:], d)
        mm_big("wT_m1o", [t[:] for t in posret], ep_store(pc[1], "m1o_b"))

        # ======== P3.6 (needs rcv2b): kv retrieval + kvo ========
        rCc = work.tile([128, V], bf16, tag="cbfc2", name="rCc")
        nc.vector.tensor_copy(rCc[:], rcv2b[:, 0:V])
        rCs = work.tile([128, V], bf16, tag="cbfs2", name="rCs")
        nc.vector.tensor_copy(rCs[:], rcv2b[:, V:2 * V])
        retr_fm = work.tile([V, NT], bf16, tag="retrfm", name="retrfm")
        for j in range(NCH):
            ch = slice(128 * j, 128 * (j + 1))
            ps_r2 = pkv.tile([128, V], f32, tag="pr")
            nc.tensor.matmul(ps_r2[:], Qc[:, ch], rCc[:],
                             start=True, stop=False)
            nc.tensor.matmul(ps_r2[:], Qs[:, ch], rCs[:],
                             start=False, stop=True)
            t = work.tile([128, V], bf16, tag="rsc", name=f"rsc{j}")
            nc.vector.tensor_tensor(t[:], ps_r2[:],
                                    retr_sb[:, V * j:V * (j + 1)], Alu.add)
            nc.vector.tensor_scalar(t[:], t[:], rstd_g_tm[:, j:j + 1],
                                    None, Alu.mult)
            ps_f = psm.tile([V, 128], bf16, tag="tr2")
            nc.tensor.transpose(ps_f[:], t[:], eyeb[:])
            nc.scalar.copy(retr_fm[:, ch], ps_f[:])
        dbg("retr_fm", retr_fm[:], 0)

        kvo_w = work.tile([V, D], bf16, tag="wkvo", name="wkvo")
        nc.sync.dma_start(out=kvo_w[:], in_=wts["wT_kvo"][:])
        for o in range(ND):
            for blk in range(NBLK):
                cs = slice(TB * blk, TB * (blk + 1))
                ps = pb.tile([128, TB], f32, tag="lin")
                nc.tensor.matmul(ps[:], kvo_w[:, 128 * o:128 * (o + 1)],
                                 retr_fm[:, cs], start=True, stop=True)
                nc.scalar.activation(pc[2][o][:, cs], ps[:], A.Identity,
                                     bias=bc("kvo_b", o))

        for p in range(5):
            for d in range(ND):
                dbg(f"pc{p}", pc[p][d][:], d)

        pkv.release()
        work.release()

        # ======== P6: LN + o1 + o2 (fresh pool) ========
        p6 = tc.alloc_tile_pool(name="p6", bufs=1)
        pst = tc.alloc_tile_pool(name="pst", bufs=1, space="PSUM")

        pieces = [pc[p][d] for p in range(5) for d in range(ND)]

        # mean: 20-matmul group per blk; copy rows to SBUF
        m_sb = p6.tile([1, NT], f32, tag="msb", name="msb")
        sq_sb = p6.tile([1, NT], f32, tag="sqsb", name="sqsb")
        ps_mean = pst.tile([1, NT], f32, tag="stat")
        for blk in range(NBLK):
            cs = slice(TB * blk, TB * (blk + 1))
            for i, pt in enumerate(pieces):
                nc.tensor.matmul(ps_mean[0:1, cs], onesb[:], pt[:, cs],
                                 start=(i == 0), stop=(i == len(pieces) - 1))
            nc.scalar.copy(m_sb[0:1, cs], ps_mean[0:1, cs])
        ps_sq = pst.tile([1, NT], f32, tag="stat")
        for blk in range(NBLK):
            cs = slice(TB * blk, TB * (blk + 1))
            for i, pt in enumerate(pieces):
                sq = p6.tile([128, TB], bf16, tag="sqbuf", bufs=2,
                             name=f"sq{blk}_{i}")
                nc.scalar.activation(sq[:], pt[:, cs], A.Square)
                nc.tensor.matmul(ps_sq[0:1, cs], onesb[:], sq[:],
                                 start=(i == 0), stop=(i == len(pieces) - 1))
            nc.scalar.copy(sq_sb[0:1, cs], ps_sq[0:1, cs])

        # transposed stats: [128, NCH] space
        mT = p6.tile([128, NCH], f32, tag="mT", name="mT")
        sqT = p6.tile([128, NCH], f32, tag="sqT", name="sqT")
        for j in range(NCH):
            psj = psm.tile([128, 2], f32, tag="tr")
            nc.tensor.transpose(psj[:, 0:1], m_sb[0:1, 128 * j:128 * (j + 1)],
                                eyef[0:1, 0:1])
            nc.tensor.transpose(psj[:, 1:2], sq_sb[0:1, 128 * j:128 * (j + 1)],
                                eyef[0:1, 0:1])
            nc.scalar.copy(mT[:, j:j + 1], psj[:, 0:1])
            nc.scalar.copy(sqT[:, j:j + 1], psj[:, 1:2])
        nc.vector.tensor_scalar(mT[:], mT[:], 1.0 / (5 * D), None, Alu.mult)
        vT = p6.tile([128, NCH], f32, tag="vT", name="vT")
        nc.vector.tensor_tensor(vT[:], mT[:], mT[:], Alu.mult)
        nc.vector.scalar_tensor_tensor(vT[:], sqT[:], 1.0 / (5 * D), vT[:],
                                       Alu.mult, Alu.subtract)
        # [rsqrt table set]
        rT = p6.tile([128, NCH], f32, tag="rT", name="rT")
        nc.scalar.activation(vT[:], vT[:], A.Ln, bias=bc("eps_ln"))
        nc.scalar.activation(rT[:], vT[:], A.Exp, scale=-0.5)
        # back to rows (bf16) for broadcast/negw matmuls
        rstd_row = p6.tile([1, NT], bf16, tag="rrow", name="rrow")
        m_row = p6.tile([1, NT], bf16, tag="mrow", name="mrow")
        for j in range(NCH):
            psj = psm.tile([1, 256], f32, tag="tr")
            nc.tensor.transpose(psj[0:1, 0:128], rT[:, j:j + 1], eyeff[:])
            nc.tensor.transpose(psj[0:1, 128:256], mT[:, j:j + 1], eyeff[:])
            nc.scalar.copy(rstd_row[0:1, 128 * j:128 * (j + 1)],
                           psj[0:1, 0:128])
            nc.scalar.copy(m_row[0:1, 128 * j:128 * (j + 1)],
                           psj[0:1, 128:256])
        dbg("ln_m", m_row[:], 0, pool=p6)

        rstd_bc = p6.tile([128, NT], bf16, tag="rstdbc", name="rstdbc")
        for blk in range(NBLK):
            cs = slice(TB * blk, TB * (blk + 1))
            psb = pb.tile([128, TB], f32, tag="lin")
            nc.tensor.matmul(psb[:], ones_r1[:], rstd_row[0:1, cs],
                             start=True, stop=True)
            nc.scalar.copy(rstd_bc[:, cs], psb[:])

        # o1 [gelu table set]
        h1 = [p6.tile([128, NT], bf16, tag=f"h1{o}", name=f"h1{o}")
              for o in range(2 * ND)]
        for o in range(2 * ND):
            o1rows = []
            for i in range(5 * ND):
                t = p6.tile([128, 128], bf16, tag="wo1", bufs=6,
                            name=f"o1r{o}_{i}")
                nc.sync.dma_start(
                    out=t[:],
                    in_=wts["wT_o1"][128 * i:128 * (i + 1),
                                     128 * o:128 * (o + 1)])
                o1rows.append(t)
            for blk in range(NBLK):
                cs = slice(TB * blk, TB * (blk + 1))
                ps = pb.tile([128, TB], f32, tag="lin")
                for i, pt in enumerate(pieces):
                    nc.tensor.matmul(ps[:], o1rows[i][:], pt[:, cs],
                                     start=(i == 0), stop=False)
                nc.tensor.matmul(ps[:],
                                 negw_sb[0:1, 128 * o:128 * (o + 1)],
                                 m_row[0:1, cs], start=False, stop=True)
                h1pre = p6.tile([128, TB], f32, tag="h1pre",
                                name=f"h1p{o}_{blk}")
                nc.vector.tensor_tensor(h1pre[:], ps[:], rstd_bc[:, cs],
                                        Alu.mult)
                nc.scalar.activation(h1[o][:, cs], h1pre[:], A.Gelu,
                                     bias=bc("o1_b", o))
        for d in range(ND):
            dbg("h1", h1[d][:], d, pool=p6)

        o2rows = []
        for i in range(2 * ND):
            t = p6.tile([128, D], bf16, tag="wo2", bufs=8, name=f"o2r{i}")
            nc.sync.dma_start(out=t[:],
                              in_=wts["wT_o2"][128 * i:128 * (i + 1), :])
            o2rows.append(t)
        o2b_sb = p6.tile([1, D], bf16, tag="o2b", name="o2b")
        nc.sync.dma_start(out=o2b_sb[:], in_=wts["o2b_row"][:])
        for j in range(NCH):
            ch = slice(128 * j, 128 * (j + 1))
            ps = pb.tile([128, D], f32, tag="lin")
            for i in range(2 * ND):
                nc.tensor.matmul(ps[:], h1[i][:, ch], o2rows[i][:],
                                 start=(i == 0), stop=False)
            nc.tensor.matmul(ps[:], ones_r1[:], o2b_sb[:],
                             start=False, stop=True)
            xres = p6.tile([128, D], f32, tag="xres", bufs=2,
                           name=f"xres{j}")
            nc.sync.dma_start(out=xres[:],
                              in_=xres_in[128 * j:128 * (j + 1), :])
            out_sb = p6.tile([128, D], f32, tag="outsb", bufs=2,
                             name=f"out{j}")
            nc.vector.tensor_tensor(out_sb[:], ps[:], xres[:], Alu.add)
            nc.sync.dma_start(out=y_out[128 * j:128 * (j + 1), :],
                              in_=out_sb[:])

        pst.release()
        p6.release()
        dram.release()
        psm.release()
        pb.release()
        keep.release()
        con.release()

    fixup_excess_waits(nc)
    return nc, dbg_shapes


# ===================== host side =====================

def _prep_host(inputs):
    import ml_dtypes
    bft = ml_dtypes.bfloat16
    g = {k: np.asarray(v, dtype=np.float32) for k, v in inputs.items()}
    c = float(np.abs(g["mag_scale"]))
    absw = np.abs(g["omega_scale"])

    def pack4(wT, width):
        return np.ascontiguousarray(
            wT.reshape(ND, 128, width).transpose(1, 0, 2).reshape(
                128, ND * width))

    W = {}
    W["wT_tw"] = (g["tw_w"] * absw[:, None]).T
    W["wT_pi0"] = g["pi0_w"].T
    W["wT_pi2"] = g["pi2_w"].T
    W["wT_m1v"] = (g["m1v_w"] * c).T
    W["wT_mag"] = g["mag_w"].T
    W["wT_qo"] = g["qo_w"].T
    W["wT_cp"] = g["cp_w"].T
    W["wT_m1o"] = (g["m1o_w"] / math.sqrt(D)).T
    W["kepack"] = pack4(g["ke_w"].T, 128)
    W["vepack"] = pack4(g["ve_w"].T, V)
    W["sgpack"] = pack4(g["sg_w"].T, 1)
    W["wT_sk0"] = g["sk0_w"].T
    W["sk2pack"] = pack4(g["sk2_w"].T, 128)
    W["wT_kvo"] = (g["kvo_w"] / math.sqrt(P)).T
    o1w = g["o1_w"] * g["ln_g"][None, :]
    W["wT_o1"] = o1w.T
    W["wT_o2"] = g["o2_w"].T
    W["o2b_row"] = g["o2_b"][None, :]
    W["negw_row"] = -o1w.sum(axis=1)[None, :]
    W["ones_col"] = np.ones((128, 1), np.float32)
    W["ones_row1"] = np.ones((1, 128), np.float32)
    W["eye_b"] = np.eye(128, dtype=np.float32)
    W["tril_b"] = np.triu(np.ones((128, 128), np.float32))
    cd = np.zeros((128, 2 * ND * K * 128), np.float32)
    for br, wname in ((0, "lc_w"), (1, "cg_w")):
        for d in range(ND):
            for k in range(K):
                off = ((br * ND + d) * K + k) * 128
                cd[:, off:off + 128] = np.diag(
                    g[wname][128 * d:128 * (d + 1), 0, k])
    W["convdiag"] = cd
    W = {k: np.ascontiguousarray(v).astype(bft) for k, v in W.items()}

    b1p = g["o1_b"] + g["o1_w"] @ g["ln_b"]

    bias = np.zeros((128, NBIAS), np.float32)
    def put(name, vec, i=0):
        v = np.asarray(vec, np.float32).ravel()
        bias[:len(v), BC[name] + i] = v
    for d in range(ND):
        sl = slice(128 * d, 128 * (d + 1))
        put("tw_b", (g["tw_b"] * absw)[sl], d)
        put("pi0_b", g["pi0_b"][sl], d)
        put("pi2_b", g["pi2_b"][sl], d)
        put("m1v_b", (g["m1v_b"] * c)[sl], d)
        put("mag_b", g["mag_b"][sl], d)
        put("qo_b", g["qo_b"][sl], d)
        put("cp_b", g["cp_b"][sl], d)
        put("m1o_b", g["m1o_b"][sl], d)
        put("sk0_b", g["sk0_b"][sl], d)
        put("kvo_b", g["kvo_b"][sl], d)
        put("lc_b", g["lc_b"][sl], d)
        put("cg_b", g["cg_b"][sl], d)
    put("ke_b", g["ke_b"])
    put("ve_b", g["ve_b"])
    put("sg_b", g["sg_b"])
    put("sk2_b", g["sk2_b"])
    for o in range(8):
        put("o1_b", b1p[128 * o:128 * (o + 1)], o)
    put("halfpi", np.full(128, HALF_PI))
    put("eps_mag", np.full(128, 1e-8))
    put("c_mag", np.full(128, c))
    put("eps_ln", np.full(128, 1e-5))

    pos = np.arange(1, L + 1, dtype=np.float32)

    x = g["x"]
    in_maps = []
    for core in range(N_CORES):
        b, h = core // 2, core % 2
        xe = np.zeros((NT + 3, D), np.float32)
        if h == 0:
            xe[3:] = x[b, 0:NT]
        else:
            xe[:] = x[b, NT - 3:2 * NT]
        xT = np.ascontiguousarray(xe.T).astype(bft)
        rp = np.broadcast_to(1.0 / pos[h * NT:(h + 1) * NT][None, :],
                             (128, NT)).astype(bft)
        m = {"xT": xT, "x_res": np.ascontiguousarray(xe[3:]),
             "bias_pack": bias, "recip_pos": rp,
             "eye_f": np.eye(V, dtype=np.float32),
             "eye_ff": np.eye(128, dtype=np.float32),
             "send_mask": np.full((128, 1), 1.0 - h, np.float32),
             "use_mask": np.full((128, 1), float(h), np.float32)}
        m.update(W)
        in_maps.append(m)
    return in_maps


_CACHE = {}

def _get_built(debug=()):
    key = tuple(sorted(debug))
    if key not in _CACHE:
        _CACHE[key] = build_nc(key)
    return _CACHE[key]


def run_cores(inputs, debug=(), trace=False):
    from concourse.bass_utils import run_bass_kernel_spmd
    nc, dbg_shapes = _get_built(debug)
    in_maps = _prep_host(inputs)
    res = run_bass_kernel_spmd(nc, in_maps, list(range(N_CORES)),
                               trace=trace)
    return res


def kernel(**inputs):
    results = run_cores(inputs).results
    out = np.empty((B, L, D), np.float32)
    for core in range(N_CORES):
        b, h = core // 2, core % 2
        out[b, h * NT:(h + 1) * NT] = results[core]["y"]
    return out
